# revision 8
# baseline (speedup 1.0000x reference)
"""PoET transformer-with-KV-prefix kernel for 8 Trainium2 NeuronCores.

Sharding: tensor-parallel over heads (2 heads/core) for attention and over
FFN columns (512/core) for the MLP.  Activations [B*L=128, D=1024] are
replicated; each block ends in an 8-core AllReduce of the output projection
partial sums.  LayerNorm gains/biases are folded into the following weight
matrices host-side, so on-device LN is a pure normalize.

All heavy matmuls run activation-stationary (lhsT = transposed activations)
with the weights streaming as the moving operand (N>=384 -> full-rate fp32r).
"""

import sys
import numpy as np

for _p in ("/opt/trn_rl_repo", "/root/.axon_site/_ro/trn_rl_repo"):
    if _p not in sys.path:
        sys.path.insert(0, _p)

import concourse.bass as bass
import concourse.bacc as bacc
import concourse.mybir as mybir
from concourse.tile import TileContext
from concourse.bass_utils import run_bass_kernel_spmd

# Problem dims (hardcoded per spec)
NL, B, L, D, H, Dh, S, DF = 2, 8, 16, 1024, 16, 64, 2048, 4096
ROPE_BASE = 10000.0
LN_EPS = 1e-5

N_CORES = 8
R = B * L            # 128 token rows
HPC = H // N_CORES   # 2 heads per core
FPC = HPC * Dh       # 128 features per core
DFC = DF // N_CORES  # 512 ffn cols per core
T = S + L            # 2064 total keys (2048 prefix + 16 new)
NT_PRE = S // 128    # 16 prefix t-tiles
NT = NT_PRE + 1      # 17 t-tiles including the new-token tile

F32 = mybir.dt.float32
F32R = mybir.dt.float32r
RG = [list(range(N_CORES))]

# Tunables
MM_REDUCED = True    # use fp32r for the big matmuls
WARMUP_CC = True     # tiny AllGather at t=0 to absorb collective setup/skew


RDT = F32R if MM_REDUCED else F32


# ---------------------------------------------------------------------------
# Host-side input prep: fold LN into weights, transpose KV, slice per core.
# ---------------------------------------------------------------------------

def _prep_in_maps(inp):
    f = lambda k: np.asarray(inp[k], dtype=np.float32)
    x = f('x').reshape(R, D)

    # rope tables (token-major): row r -> position S + r % L
    pos = (S + np.arange(R) % L).astype(np.float32)
    inv = ROPE_BASE ** (-np.arange(Dh // 2, dtype=np.float32) / (Dh // 2))
    ang = pos[:, None] * inv[None, :]              # [128, 32]
    cos32, sin32 = np.cos(ang), np.sin(ang)
    blk_cos = np.concatenate([cos32, cos32], 1)    # [128, 64]
    blk_ssin = np.concatenate([-sin32, sin32], 1)  # [128, 64]
    cos2 = np.tile(blk_cos, (1, 4)).astype(np.float32)    # [128, 256] (q_h0,q_h1,k_h0,k_h1)
    ssin2 = np.tile(blk_ssin, (1, 4)).astype(np.float32)

    # block-diagonal own-batch mask for the new-token scores
    mask01 = np.kron(np.eye(B, dtype=np.float32), np.ones((L, L), np.float32))

    shared = {'x': x, 'cos2': cos2, 'ssin2': ssin2, 'mask01': mask01}

    # attention blocks in execution layer order: (layer, kind)
    attn_specs = [(0, 'self'), (0, 'cross'), (1, 'self'), (1, 'cross')]
    per_core = [dict(shared) for _ in range(N_CORES)]

    for bi, (l, kind) in enumerate(attn_specs):
        g = f('ln1_g' if kind == 'self' else 'ln2_g')[l]
        be = f('ln1_b' if kind == 'self' else 'ln2_b')[l]
        Wq, Wk, Wv, Wo = (f(f'{kind}_W{m}')[l] for m in 'qkvo')
        k_mem = f(f'{kind}_k_mem')[l]   # [S, H, Dh]
        v_mem = f(f'{kind}_v_mem')[l]
        Wq_e, Wk_e, Wv_e = g[:, None] * Wq, g[:, None] * Wk, g[:, None] * Wv
        bq, bk, bv = be @ Wq, be @ Wk, be @ Wv   # [D]
        for c in range(N_CORES):
            cs = slice(c * FPC, (c + 1) * FPC)
            wqkv = np.concatenate([Wq_e[:, cs], Wk_e[:, cs], Wv_e[:, cs]], 1)  # [1024, 384]
            bqkv = np.concatenate([bq[cs], bk[cs], bv[cs]])                    # [384]
            m = per_core[c]
            m[f'wqkv{bi}'] = np.ascontiguousarray(wqkv.reshape(8, 128, 3 * FPC))
            m[f'bqkv{bi}'] = np.ascontiguousarray(np.tile(bqkv[None, :], (128, 1)))
            m[f'wo{bi}'] = np.ascontiguousarray(Wo[cs, :])                     # [128, 1024]
            # K^T per head, feature-major: [128 (2h x 64), S]
            kt = k_mem[:, 2 * c:2 * c + 2, :].transpose(1, 2, 0).reshape(FPC, S)
            m[f'kt{bi}'] = np.ascontiguousarray(kt)
            # V token-major tiles: [128 (tok%128), 16*128 (ttile, 2h x 64)]
            v = v_mem[:, 2 * c:2 * c + 2, :].reshape(NT_PRE, 128, FPC)
            m[f'v{bi}'] = np.ascontiguousarray(v.transpose(1, 0, 2).reshape(128, NT_PRE * FPC))

    for l in range(NL):
        g3, b3 = f('ln3_g')[l], f('ln3_b')[l]
        W1, b1, W2, b2 = f('W1')[l], f('b1')[l], f('W2')[l], f('b2')[l]
        W1_e = g3[:, None] * W1
        b1_e = b1 + b3 @ W1
        for c in range(N_CORES):
            cs = slice(c * DFC, (c + 1) * DFC)
            m = per_core[c]
            m[f'w1_{l}'] = np.ascontiguousarray(W1_e[:, cs].reshape(8, 128, DFC))
            m[f'b1_{l}'] = np.ascontiguousarray(np.tile(b1_e[None, cs], (128, 1)))
            m[f'w2_{l}'] = np.ascontiguousarray(W2[cs, :].reshape(4, 128, D))
            m[f'b2_{l}'] = np.ascontiguousarray(np.tile(b2[None, :] / N_CORES, (128, 1)))
    return per_core


# ---------------------------------------------------------------------------
# Device program (SPMD; identical on all cores, per-core data via in_maps)
# ---------------------------------------------------------------------------

def _build():
    from concourse import masks

    nc = bacc.Bacc("TRN2", target_bir_lowering=False, debug=False,
                   num_devices=N_CORES)
    P = {}
    P['x'] = nc.declare_dram_parameter('x', [R, D], F32, isOutput=False)
    P['cos2'] = nc.declare_dram_parameter('cos2', [R, 256], F32, isOutput=False)
    P['ssin2'] = nc.declare_dram_parameter('ssin2', [R, 256], F32, isOutput=False)
    P['mask01'] = nc.declare_dram_parameter('mask01', [R, R], F32, isOutput=False)
    for bi in range(4):
        P[f'wqkv{bi}'] = nc.declare_dram_parameter(f'wqkv{bi}', [8, 128, 3 * FPC], RDT, isOutput=False)
        P[f'bqkv{bi}'] = nc.declare_dram_parameter(f'bqkv{bi}', [R, 3 * FPC], F32, isOutput=False)
        P[f'wo{bi}'] = nc.declare_dram_parameter(f'wo{bi}', [FPC, D], RDT, isOutput=False)
        P[f'kt{bi}'] = nc.declare_dram_parameter(f'kt{bi}', [FPC, S], RDT, isOutput=False)
        P[f'v{bi}'] = nc.declare_dram_parameter(f'v{bi}', [128, NT_PRE * FPC], F32, isOutput=False)
    for l in range(NL):
        P[f'w1_{l}'] = nc.declare_dram_parameter(f'w1_{l}', [8, 128, DFC], RDT, isOutput=False)
        P[f'b1_{l}'] = nc.declare_dram_parameter(f'b1_{l}', [R, DFC], F32, isOutput=False)
        P[f'w2_{l}'] = nc.declare_dram_parameter(f'w2_{l}', [4, 128, D], RDT, isOutput=False)
        P[f'b2_{l}'] = nc.declare_dram_parameter(f'b2_{l}', [R, D], F32, isOutput=False)
    out = nc.declare_dram_parameter('out', [R, D], F32, isOutput=True)

    with TileContext(nc) as tc:
        with (
            tc.tile_pool(name="cpool", bufs=1) as cpool,
            tc.tile_pool(name="hpool", bufs=2) as hpool,
            tc.tile_pool(name="qpool", bufs=2) as qpool,
            tc.tile_pool(name="apool", bufs=2) as apool,
            tc.tile_pool(name="kvpool", bufs=2) as kvpool,
            tc.tile_pool(name="wpool", bufs=3) as wpool,
            tc.tile_pool(name="spool", bufs=4) as spool,
            tc.tile_pool(name="ppt", bufs=3, space="PSUM") as ppt,
            tc.tile_pool(name="pps", bufs=2, space="PSUM") as pps,
            tc.tile_pool(name="ppa", bufs=2, space="PSUM") as ppa,
            tc.tile_pool(name="dpool", bufs=2, space="DRAM") as dpool,
        ):
            ident = cpool.tile([128, 128], F32, tag="ident")
            masks.make_identity(nc, ident[:, :])
            x_sb = cpool.tile([R, D], F32, tag="x")
            nc.sync.dma_start(x_sb[:, :], P['x'][:, :])
            cos2 = cpool.tile([R, 256], F32, tag="cos2")
            nc.sync.dma_start(cos2[:, :], P['cos2'][:, :])
            ssin2 = cpool.tile([R, 256], F32, tag="ssin2")
            nc.sync.dma_start(ssin2[:, :], P['ssin2'][:, :])
            mask01 = cpool.tile([R, R], F32, tag="mask01")
            nc.sync.dma_start(mask01[:, :], P['mask01'][:, :])
            eps_t = cpool.tile([128, 1], F32, tag="eps")
            nc.gpsimd.memset(eps_t[:, :], LN_EPS)

            if WARMUP_CC:
                wu_in = dpool.tile([2, 16], F32, tag="wu_in")
                wu_out = dpool.tile([16, 16], F32, tag="wu_out")
                nc.gpsimd.dma_start(wu_in[:], P['x'][0:2, 0:16])
                nc.gpsimd.collective_compute(
                    "AllGather", mybir.AluOpType.bypass, replica_groups=RG,
                    ins=[wu_in.opt()], outs=[wu_out.opt()])

            def layer_norm(tag):
                """x_sb -> h [R, D] (pure normalize; gains folded into weights)."""
                sums = spool.tile([R, 8], F32, tag="lnsums")
                nc.vector.tensor_reduce(sums[:, 0:1], x_sb[:, :],
                                        axis=mybir.AxisListType.X, op=mybir.AluOpType.add)
                sq = hpool.tile([R, D], F32, tag="sq")
                nc.scalar.activation(sq[:, :], x_sb[:, :],
                                     mybir.ActivationFunctionType.Square,
                                     accum_out=sums[:, 1:2])
                # mean, E[x^2], var = E[x^2] - mean^2
                nc.scalar.mul(sums[:, 2:3], sums[:, 0:1], 1.0 / D)          # mean
                nc.vector.tensor_tensor(sums[:, 3:4], sums[:, 2:3], sums[:, 2:3],
                                        op=mybir.AluOpType.mult)            # mean^2
                nc.vector.tensor_scalar(sums[:, 4:5], sums[:, 1:2], 1.0 / D,
                                        None, op0=mybir.AluOpType.mult)     # E[x^2]
                nc.vector.tensor_tensor(sums[:, 5:6], sums[:, 4:5], sums[:, 3:4],
                                        op=mybir.AluOpType.subtract)        # var
                nc.scalar.activation(sums[:, 6:7], sums[:, 5:6],
                                     mybir.ActivationFunctionType.Sqrt, bias=eps_t[:, 0:1])
                nc.vector.reciprocal(sums[:, 7:8], sums[:, 6:7])            # rstd
                h = hpool.tile([R, D], F32, tag="h")
                nc.vector.tensor_scalar(h[:, :], x_sb[:, :], sums[:, 2:3], sums[:, 7:8],
                                        op0=mybir.AluOpType.subtract,
                                        op1=mybir.AluOpType.mult)
                return h

            def transpose_128(src_ap, dst_ap):
                """PE-transpose one [128, <=128] slice into SBUF dst."""
                p = ppt.tile([128, 128], F32, tag="tp")
                np_, nf = src_ap.shape[0], src_ap.shape[1]
                nc.tensor.transpose(p[:nf, :np_], src_ap, ident[:np_, :np_])
                nc.vector.tensor_copy(dst_ap, p[:nf, :np_])

            def transpose_big(src, dst, n):
                for i in range(n):
                    transpose_128(src[:, 128 * i:128 * (i + 1)],
                                  dst[:, 128 * i:128 * (i + 1)])

            def all_reduce_add(y_sb, tag=""):
                """DMA partial [R, D] SBUF through an AllReduce, add into x_sb."""
                cin = dpool.tile([R, D], F32, tag="cc_in")
                cout = dpool.tile([R, D], F32, tag="cc_out")
                nc.gpsimd.dma_start(cin[:, :], y_sb[:, :])
                nc.gpsimd.collective_compute(
                    "AllReduce", mybir.AluOpType.add, replica_groups=RG,
                    ins=[cin.opt()], outs=[cout.opt()])
                y = hpool.tile([R, D], F32, tag="yred")
                nc.sync.dma_start(y[:, :], cout[:, :])
                nc.vector.tensor_add(x_sb[:, :], x_sb[:, :], y[:, :])

            def attn_block(bi):
                h = layer_norm(f"a{bi}")
                hT = hpool.tile([R, D], RDT, tag="hT")
                transpose_big(h, hT, 8)

                # qkv = h @ Wqkv_c + bqkv   [R, 384] (token-major)
                qkv_ps = pps.tile([R, 3 * FPC], F32, tag="ps512")
                for kt_i in range(8):
                    w = wpool.tile([128, 3 * FPC], RDT, tag="wqkv")
                    nc.sync.dma_start(w[:, :], P[f'wqkv{bi}'][kt_i])
                    nc.tensor.matmul(qkv_ps[:, :],
                                     (hT[:, 128 * kt_i:128 * (kt_i + 1)]),
                                     (w[:, :]),
                                     start=(kt_i == 0), stop=(kt_i == 7))
                bq = wpool.tile([R, 3 * FPC], F32, tag="bqkv")
                nc.sync.dma_start(bq[:, :], P[f'bqkv{bi}'][:, :])
                qkv = qpool.tile([R, 3 * FPC], F32, tag="qkv")
                nc.vector.tensor_add(qkv[:, :], qkv_ps[:, :], bq[:, :])

                # rope on q|k region [R, 256]
                tmp = qpool.tile([R, 256], F32, tag="ropetmp")
                for blk in range(4):
                    a0, a1, a2 = 64 * blk, 64 * blk + 32, 64 * blk + 64
                    nc.vector.tensor_mul(tmp[:, a0:a1], qkv[:, a1:a2], ssin2[:, a0:a1])
                    nc.vector.tensor_mul(tmp[:, a1:a2], qkv[:, a0:a1], ssin2[:, a1:a2])
                qk_r = qpool.tile([R, 256], F32, tag="qkr")
                nc.vector.tensor_mul(qk_r[:, :], qkv[:, 0:256], cos2[:, :])
                nc.vector.tensor_add(qk_r[:, :], qk_r[:, :], tmp[:, :])

                qT = qpool.tile([FPC, R], RDT, tag="qT")
                transpose_128(qk_r[:, 0:128], qT[:, :])
                kTn = qpool.tile([FPC, R], RDT, tag="kTn")
                transpose_128(qk_r[:, 128:256], kTn[:, :])

                # prefix K^T and V
                kt_sb = kvpool.tile([FPC, S], RDT, tag="kt")
                nc.sync.dma_start(kt_sb[:, :], P[f'kt{bi}'][:, :])
                v_sb = kvpool.tile([128, NT_PRE * FPC], F32, tag="v")
                nc.sync.dma_start(v_sb[:, :], P[f'v{bi}'][:, :])

                O = qpool.tile([R, FPC], F32, tag="O")
                for hh in range(HPC):
                    hs = slice(Dh * hh, Dh * (hh + 1))
                    A = apool.tile([R, S + R], F32, tag="A")
                    sums = spool.tile([R, 8], F32, tag="smsums")
                    # prefix scores -> exp, 4 chunks of 512
                    for j in range(4):
                        s_ps = pps.tile([R, 512], F32, tag="ps512")
                        nc.tensor.matmul(s_ps[:, :],
                                         (qT[hs, :]),
                                         (kt_sb[hs, 512 * j:512 * (j + 1)]),
                                         start=True, stop=True)
                        nc.scalar.activation(A[:, 512 * j:512 * (j + 1)], s_ps[:, :],
                                             mybir.ActivationFunctionType.Exp,
                                             scale=1.0 / np.sqrt(Dh),
                                             accum_out=sums[:, j:j + 1])
                    # new-token scores (full [R, R], then block-diag mask)
                    sn_ps = ppt.tile([128, 128], F32, tag="tp")
                    nc.tensor.matmul(sn_ps[:, :], (qT[hs, :]), (kTn[hs, :]),
                                     start=True, stop=True)
                    en = qpool.tile([R, R], F32, tag="expn")
                    nc.scalar.activation(en[:, :], sn_ps[:, :],
                                         mybir.ActivationFunctionType.Exp,
                                         scale=1.0 / np.sqrt(Dh))
                    nc.vector.tensor_mul(A[:, S:S + R], en[:, :], mask01[:, :])
                    nc.vector.tensor_reduce(sums[:, 4:5], A[:, S:S + R],
                                            axis=mybir.AxisListType.X, op=mybir.AluOpType.add)
                    nc.vector.tensor_reduce(sums[:, 5:6], sums[:, 0:5],
                                            axis=mybir.AxisListType.X, op=mybir.AluOpType.add)
                    nc.vector.reciprocal(sums[:, 6:7], sums[:, 5:6])

                    # A^T tiles + AV accumulation over 17 k-tiles
                    AT = apool.tile([128, NT * 128], F32, tag="AT")
                    av_ps = ppa.tile([R, Dh], F32, tag="av")
                    for t in range(NT):
                        transpose_128(A[:, 128 * t:128 * (t + 1)],
                                      AT[:, 128 * t:128 * (t + 1)])
                        if t < NT_PRE:
                            rhs = v_sb[:, FPC * t + Dh * hh: FPC * t + Dh * (hh + 1)]
                        else:
                            rhs = qkv[:, 256 + Dh * hh: 256 + Dh * (hh + 1)]
                        nc.tensor.matmul(av_ps[:, :],
                                         AT[:, 128 * t:128 * (t + 1)], rhs,
                                         start=(t == 0), stop=(t == NT - 1))
                    nc.vector.tensor_scalar(O[:, Dh * hh:Dh * (hh + 1)], av_ps[:, :],
                                            sums[:, 6:7], None, op0=mybir.AluOpType.mult)

                OT = qpool.tile([FPC, R], RDT, tag="OT")
                transpose_128(O[:, :], OT[:, :])
                wo = wpool.tile([FPC, D], RDT, tag="wo")
                nc.sync.dma_start(wo[:, :], P[f'wo{bi}'][:, :])
                y_attn = qpool.tile([R, D], F32, tag="y2")
                for j in range(2):
                    y_ps = pps.tile([R, 512], F32, tag="ps512")
                    nc.tensor.matmul(y_ps[:, :], (OT[:, :]),
                                     (wo[:, 512 * j:512 * (j + 1)]),
                                     start=True, stop=True)
                    nc.vector.tensor_copy(y_attn[:, 512 * j:512 * (j + 1)], y_ps[:, :])
                all_reduce_add(y_attn, tag=f"a{bi}")

            def mlp_block(l):
                h = layer_norm(f"m{l}")
                hT = hpool.tile([R, D], RDT, tag="hT")
                transpose_big(h, hT, 8)

                a_ps = pps.tile([R, DFC], F32, tag="ps512")
                for kt_i in range(8):
                    w = wpool.tile([128, DFC], RDT, tag="w1")
                    nc.sync.dma_start(w[:, :], P[f'w1_{l}'][kt_i])
                    nc.tensor.matmul(a_ps[:, :],
                                     (hT[:, 128 * kt_i:128 * (kt_i + 1)]),
                                     (w[:, :]),
                                     start=(kt_i == 0), stop=(kt_i == 7))
                b1 = wpool.tile([R, DFC], F32, tag="b1")
                nc.sync.dma_start(b1[:, :], P[f'b1_{l}'][:, :])
                ab = qpool.tile([R, DFC], F32, tag="ab")
                nc.vector.tensor_add(ab[:, :], a_ps[:, :], b1[:, :])
                ag = qpool.tile([R, DFC], F32, tag="ag")
                nc.scalar.activation(ag[:, :], ab[:, :],
                                     mybir.ActivationFunctionType.Gelu_apprx_tanh)
                aT = hpool.tile([128, DFC], RDT, tag="aT")
                transpose_big(ag, aT, 4)

                y_ps = [pps.tile([R, 512], F32, tag="ps512", name=f"y2ps{j}")
                        for j in range(2)]
                for kt_i in range(4):
                    w = wpool.tile([128, D], RDT, tag="w2")
                    nc.sync.dma_start(w[:, :], P[f'w2_{l}'][kt_i])
                    for j in range(2):
                        nc.tensor.matmul(y_ps[j][:, :],
                                         (aT[:, 128 * kt_i:128 * (kt_i + 1)]),
                                         (w[:, 512 * j:512 * (j + 1)]),
                                         start=(kt_i == 0), stop=(kt_i == 3))
                b2 = wpool.tile([R, D], F32, tag="b2")
                nc.sync.dma_start(b2[:, :], P[f'b2_{l}'][:, :])
                y2 = qpool.tile([R, D], F32, tag="y2")
                for j in range(2):
                    nc.vector.tensor_add(y2[:, 512 * j:512 * (j + 1)], y_ps[j][:, :],
                                         b2[:, 512 * j:512 * (j + 1)])
                all_reduce_add(y2, tag=f"m{l}")

            for l in range(NL):
                attn_block(2 * l)
                attn_block(2 * l + 1)
                mlp_block(l)

            nc.sync.dma_start(out[:, :], x_sb[:, :])

    nc.compile()
    return nc


_cached_nc = None


def _get_nc():
    global _cached_nc
    if _cached_nc is None:
        _cached_nc = _build()
    return _cached_nc


def _run(inputs, trace=False):
    nc = _get_nc()
    in_maps = _prep_in_maps(inputs)
    res = run_bass_kernel_spmd(nc, in_maps, list(range(N_CORES)), trace=trace)
    y = res.results[0]['out'].reshape(B, L, D).astype(np.float32)
    return y, res


def kernel(**inputs):
    y, _ = _run(inputs, trace=False)
    return y


# revision 10
# speedup vs baseline: 1.0263x; 1.0263x over previous
"""PoET transformer-with-KV-prefix kernel for 8 Trainium2 NeuronCores.

Sharding: tensor-parallel over heads (2 heads/core) for attention and over
FFN columns (512/core) for the MLP.  Activations [B*L=128, D=1024] are
replicated; each block ends in an 8-core AllReduce (bf16) of the output
projection partial sums.  LayerNorm gains/biases are folded into the
following weight matrices host-side, so on-device LN is a pure normalize.

Matmul dtype strategy:
 - x-stream matmuls (qkv / out-proj / mlp): fp32r activations-stationary,
   weights moving with N>=384 (full-rate fp32r).
 - attention-internal path (scores / softmax / A@V): bf16 (q, k, K-prefix,
   A, V), fp32 PSUM accumulation.  The V tiles carry a ones-column so the
   A@V accumulation also produces the softmax denominator for free.
"""

import sys
import numpy as np

for _p in ("/opt/trn_rl_repo", "/root/.axon_site/_ro/trn_rl_repo"):
    if _p not in sys.path:
        sys.path.insert(0, _p)

import ml_dtypes
import concourse.bass as bass
import concourse.bacc as bacc
import concourse.mybir as mybir
from concourse.tile import TileContext
from concourse.bass_utils import run_bass_kernel_spmd

# Problem dims (hardcoded per spec)
NL, B, L, D, H, Dh, S, DF = 2, 8, 16, 1024, 16, 64, 2048, 4096
ROPE_BASE = 10000.0
LN_EPS = 1e-5

N_CORES = 8
R = B * L            # 128 token rows
HPC = H // N_CORES   # 2 heads per core
FPC = HPC * Dh       # 128 features per core
DFC = DF // N_CORES  # 512 ffn cols per core
NT_PRE = S // 128    # 16 prefix t-tiles
NT = NT_PRE + 1      # 17 t-tiles including the new-token tile

F32 = mybir.dt.float32
F32R = mybir.dt.float32r
BF16 = mybir.dt.bfloat16
NPBF = ml_dtypes.bfloat16
RG = [list(range(N_CORES))]

WARMUP_CC = True     # tiny AllGather at t=0 to absorb collective setup/skew
RDT = F32R


# ---------------------------------------------------------------------------
# Host-side input prep: fold LN into weights, transpose KV, slice per core.
# ---------------------------------------------------------------------------

def _prep_in_maps(inp):
    f = lambda k: np.asarray(inp[k], dtype=np.float32)
    x = f('x').reshape(R, D)

    # rope tables (token-major): row r -> position S + r % L
    pos = (S + np.arange(R) % L).astype(np.float32)
    inv = ROPE_BASE ** (-np.arange(Dh // 2, dtype=np.float32) / (Dh // 2))
    ang = pos[:, None] * inv[None, :]              # [128, 32]
    cos32, sin32 = np.cos(ang), np.sin(ang)
    blk_cos = np.concatenate([cos32, cos32], 1)    # [128, 64]
    blk_ssin = np.concatenate([-sin32, sin32], 1)  # [128, 64]
    cos2 = np.tile(blk_cos, (1, 4)).astype(np.float32)    # [128, 256] (q_h0,q_h1,k_h0,k_h1)
    ssin2 = np.tile(blk_ssin, (1, 4)).astype(np.float32)

    # block-diagonal own-batch mask for the new-token scores
    mask01 = np.kron(np.eye(B, dtype=np.float32),
                     np.ones((L, L), np.float32)).astype(NPBF)

    shared = {'x': x, 'cos2': cos2, 'ssin2': ssin2, 'mask01': mask01}

    attn_specs = [(0, 'self'), (0, 'cross'), (1, 'self'), (1, 'cross')]
    per_core = [dict(shared) for _ in range(N_CORES)]

    for bi, (l, kind) in enumerate(attn_specs):
        g = f('ln1_g' if kind == 'self' else 'ln2_g')[l]
        be = f('ln1_b' if kind == 'self' else 'ln2_b')[l]
        Wq, Wk, Wv, Wo = (f(f'{kind}_W{m}')[l] for m in 'qkvo')
        k_mem = f(f'{kind}_k_mem')[l]   # [S, H, Dh]
        v_mem = f(f'{kind}_v_mem')[l]
        Wq_e, Wk_e, Wv_e = g[:, None] * Wq, g[:, None] * Wk, g[:, None] * Wv
        bq, bk, bv = be @ Wq, be @ Wk, be @ Wv   # [D]
        for c in range(N_CORES):
            cs = slice(c * FPC, (c + 1) * FPC)
            wqkv = np.concatenate([Wq_e[:, cs], Wk_e[:, cs], Wv_e[:, cs]], 1)  # [1024, 384]
            bqkv = np.concatenate([bq[cs], bk[cs], bv[cs]])                    # [384]
            m = per_core[c]
            m[f'wqkv{bi}'] = np.ascontiguousarray(wqkv.reshape(8, 128, 3 * FPC))
            m[f'bqkv{bi}'] = np.ascontiguousarray(np.tile(bqkv[None, :], (128, 1)))
            m[f'wo{bi}'] = np.ascontiguousarray(Wo[cs, :])                     # [128, 1024]
            # K^T per head, feature-major: [128 (2h x 64), S], bf16
            kt = k_mem[:, 2 * c:2 * c + 2, :].transpose(1, 2, 0).reshape(FPC, S)
            m[f'kt{bi}'] = np.ascontiguousarray(kt).astype(NPBF)
            # V token-major tiles with ones-columns: [128, 16*130], bf16
            # col layout per t-tile: [v_h0 (64) | 1 | v_h1 (64) | 1]
            v = v_mem[:, 2 * c:2 * c + 2, :].reshape(NT_PRE, 128, 2, Dh)
            va = np.ones((128, NT_PRE, 2, Dh + 1), np.float32)
            va[:, :, :, :Dh] = v.transpose(1, 0, 2, 3)
            m[f'v{bi}'] = np.ascontiguousarray(va.reshape(128, NT_PRE * 130)).astype(NPBF)

    for l in range(NL):
        g3, b3 = f('ln3_g')[l], f('ln3_b')[l]
        W1, b1, W2, b2 = f('W1')[l], f('b1')[l], f('W2')[l], f('b2')[l]
        W1_e = g3[:, None] * W1
        b1_e = b1 + b3 @ W1
        for c in range(N_CORES):
            cs = slice(c * DFC, (c + 1) * DFC)
            m = per_core[c]
            m[f'w1_{l}'] = np.ascontiguousarray(W1_e[:, cs].reshape(8, 128, DFC))
            m[f'b1_{l}'] = np.ascontiguousarray(np.tile(b1_e[None, cs], (128, 1)))
            m[f'w2_{l}'] = np.ascontiguousarray(W2[cs, :].reshape(4, 128, D))
            m[f'b2_{l}'] = np.ascontiguousarray(np.tile(b2[None, :] / N_CORES, (128, 1)))
    return per_core


# ---------------------------------------------------------------------------
# Device program (SPMD; identical on all cores, per-core data via in_maps)
# ---------------------------------------------------------------------------

def _build():
    from concourse import masks

    nc = bacc.Bacc("TRN2", target_bir_lowering=False, debug=False,
                   num_devices=N_CORES)
    P = {}
    P['x'] = nc.declare_dram_parameter('x', [R, D], F32, isOutput=False)
    P['cos2'] = nc.declare_dram_parameter('cos2', [R, 256], F32, isOutput=False)
    P['ssin2'] = nc.declare_dram_parameter('ssin2', [R, 256], F32, isOutput=False)
    P['mask01'] = nc.declare_dram_parameter('mask01', [R, R], BF16, isOutput=False)
    for bi in range(4):
        P[f'wqkv{bi}'] = nc.declare_dram_parameter(f'wqkv{bi}', [8, 128, 3 * FPC], RDT, isOutput=False)
        P[f'bqkv{bi}'] = nc.declare_dram_parameter(f'bqkv{bi}', [R, 3 * FPC], F32, isOutput=False)
        P[f'wo{bi}'] = nc.declare_dram_parameter(f'wo{bi}', [FPC, D], RDT, isOutput=False)
        P[f'kt{bi}'] = nc.declare_dram_parameter(f'kt{bi}', [FPC, S], BF16, isOutput=False)
        P[f'v{bi}'] = nc.declare_dram_parameter(f'v{bi}', [128, NT_PRE * 130], BF16, isOutput=False)
    for l in range(NL):
        P[f'w1_{l}'] = nc.declare_dram_parameter(f'w1_{l}', [8, 128, DFC], RDT, isOutput=False)
        P[f'b1_{l}'] = nc.declare_dram_parameter(f'b1_{l}', [R, DFC], F32, isOutput=False)
        P[f'w2_{l}'] = nc.declare_dram_parameter(f'w2_{l}', [4, 128, D], RDT, isOutput=False)
        P[f'b2_{l}'] = nc.declare_dram_parameter(f'b2_{l}', [R, D], F32, isOutput=False)
    out = nc.declare_dram_parameter('out', [R, D], F32, isOutput=True)

    with TileContext(nc) as tc:
        with (
            tc.tile_pool(name="cpool", bufs=1) as cpool,
            tc.tile_pool(name="hpool", bufs=2) as hpool,
            tc.tile_pool(name="qpool", bufs=2) as qpool,
            tc.tile_pool(name="apool", bufs=2) as apool,
            tc.tile_pool(name="kvpool", bufs=2) as kvpool,
            tc.tile_pool(name="wpool", bufs=3) as wpool,
            tc.tile_pool(name="spool", bufs=4) as spool,
            tc.tile_pool(name="ppt", bufs=2, space="PSUM") as ppt,
            tc.tile_pool(name="pps", bufs=2, space="PSUM") as pps,
            tc.tile_pool(name="ppa", bufs=2, space="PSUM") as ppa,
            tc.tile_pool(name="dpool", bufs=2, space="DRAM") as dpool,
        ):
            ident = cpool.tile([128, 128], F32, tag="ident")
            masks.make_identity(nc, ident[:, :])
            identb = cpool.tile([128, 128], BF16, tag="identb")
            masks.make_identity(nc, identb[:, :])
            x_sb = cpool.tile([R, D], F32, tag="x")
            nc.sync.dma_start(x_sb[:, :], P['x'][:, :])
            cos2 = cpool.tile([R, 256], F32, tag="cos2")
            nc.sync.dma_start(cos2[:, :], P['cos2'][:, :])
            ssin2 = cpool.tile([R, 256], F32, tag="ssin2")
            nc.sync.dma_start(ssin2[:, :], P['ssin2'][:, :])
            mask01 = cpool.tile([R, R], BF16, tag="mask01")
            nc.sync.dma_start(mask01[:, :], P['mask01'][:, :])
            eps_t = cpool.tile([128, 1], F32, tag="eps")
            nc.gpsimd.memset(eps_t[:, :], LN_EPS)

            if WARMUP_CC:
                wu_in = dpool.tile([2, 16], F32, tag="wu_in")
                wu_out = dpool.tile([16, 16], F32, tag="wu_out")
                nc.gpsimd.dma_start(wu_in[:], P['x'][0:2, 0:16])
                nc.gpsimd.collective_compute(
                    "AllGather", mybir.AluOpType.bypass, replica_groups=RG,
                    ins=[wu_in.opt()], outs=[wu_out.opt()])

            def layer_norm(tag):
                """x_sb -> h [R, D] fp32 (pure normalize; gains folded away)."""
                sums = spool.tile([R, 8], F32, tag="lnsums")
                nc.vector.tensor_reduce(sums[:, 0:1], x_sb[:, :],
                                        axis=mybir.AxisListType.X, op=mybir.AluOpType.add)
                sq = hpool.tile([R, D], F32, tag="sq")
                nc.scalar.activation(sq[:, :], x_sb[:, :],
                                     mybir.ActivationFunctionType.Square,
                                     accum_out=sums[:, 1:2])             # sum(x^2)
                nc.vector.tensor_scalar(sums[:, 2:3], sums[:, 0:1], 1.0 / D,
                                        None, op0=mybir.AluOpType.mult)  # mean
                nc.vector.tensor_tensor(sums[:, 3:4], sums[:, 2:3], sums[:, 2:3],
                                        op=mybir.AluOpType.mult)         # mean^2
                nc.vector.tensor_scalar(sums[:, 4:5], sums[:, 1:2], 1.0 / D,
                                        sums[:, 3:4], op0=mybir.AluOpType.mult,
                                        op1=mybir.AluOpType.subtract)    # var
                nc.scalar.activation(sums[:, 5:6], sums[:, 4:5],
                                     mybir.ActivationFunctionType.Sqrt,
                                     bias=eps_t[:, 0:1])
                nc.vector.reciprocal(sums[:, 6:7], sums[:, 5:6])         # rstd
                h = hpool.tile([R, D], F32, tag="h")
                nc.vector.tensor_scalar(h[:, :], x_sb[:, :], sums[:, 2:3], sums[:, 6:7],
                                        op0=mybir.AluOpType.subtract,
                                        op1=mybir.AluOpType.mult)
                return h

            def transpose_128(src_ap, dst_ap, bf=False):
                """PE-transpose one [128, <=128] slice into SBUF dst."""
                np_, nf = src_ap.shape[0], src_ap.shape[1]
                if bf:
                    p = ppt.tile([128, 128], BF16, tag="tpb")
                    nc.tensor.transpose(p[:nf, :np_], src_ap, identb[:np_, :np_])
                else:
                    p = ppt.tile([128, 128], F32, tag="tp")
                    nc.tensor.transpose(p[:nf, :np_], src_ap, ident[:np_, :np_])
                nc.vector.tensor_copy(dst_ap, p[:nf, :np_])

            def transpose_hT(h):
                hT = hpool.tile([R, D], RDT, tag="hT")
                for i in range(8):
                    transpose_128(h[:, 128 * i:128 * (i + 1)],
                                  hT[:, 128 * i:128 * (i + 1)])
                return hT

            def all_reduce_add(y_sb, tag=""):
                """AllReduce (bf16) the [R, D] partial and add into x_sb."""
                cin = dpool.tile([R, D], BF16, tag="cc_in")
                cout = dpool.tile([R, D], BF16, tag="cc_out")
                nc.gpsimd.dma_start(cin[:, :], y_sb[:, :])
                nc.gpsimd.collective_compute(
                    "AllReduce", mybir.AluOpType.add, replica_groups=RG,
                    ins=[cin.opt()], outs=[cout.opt()])
                y = hpool.tile([R, D], BF16, tag="yred")
                nc.sync.dma_start(y[:, :], cout[:, :])
                nc.vector.tensor_add(x_sb[:, :], x_sb[:, :], y[:, :])

            def attn_block(bi):
                # prefix K^T and V (prefetchable, no deps)
                kt_sb = kvpool.tile([FPC, S], BF16, tag="kt")
                nc.sync.dma_start(kt_sb[:, :], P[f'kt{bi}'][:, :])
                v_sb = kvpool.tile([128, NT_PRE * 130], BF16, tag="v")
                nc.sync.dma_start(v_sb[:, :], P[f'v{bi}'][:, :])

                h = layer_norm(f"a{bi}")
                hT = transpose_hT(h)

                # qkv = h @ Wqkv_c + bqkv   [R, 384] (token-major)
                qkv_ps = pps.tile([R, 3 * FPC], F32, tag="ps512")
                for kt_i in range(8):
                    w = wpool.tile([128, 3 * FPC], RDT, tag="wqkv")
                    nc.sync.dma_start(w[:, :], P[f'wqkv{bi}'][kt_i])
                    nc.tensor.matmul(qkv_ps[:, :],
                                     hT[:, 128 * kt_i:128 * (kt_i + 1)],
                                     w[:, :],
                                     start=(kt_i == 0), stop=(kt_i == 7))
                bq = wpool.tile([R, 3 * FPC], F32, tag="bqkv")
                nc.sync.dma_start(bq[:, :], P[f'bqkv{bi}'][:, :])
                qkv = qpool.tile([R, 3 * FPC], F32, tag="qkv")
                nc.vector.tensor_add(qkv[:, :], qkv_ps[:, :], bq[:, :])

                # rope on q|k region [R, 256] -> bf16
                tmp = qpool.tile([R, 256], F32, tag="ropetmp")
                for blk in range(4):
                    a0, a1, a2 = 64 * blk, 64 * blk + 32, 64 * blk + 64
                    nc.vector.tensor_mul(tmp[:, a0:a1], qkv[:, a1:a2], ssin2[:, a0:a1])
                    nc.vector.tensor_mul(tmp[:, a1:a2], qkv[:, a0:a1], ssin2[:, a1:a2])
                qk_c = qpool.tile([R, 256], F32, tag="qkc")
                nc.vector.tensor_mul(qk_c[:, :], qkv[:, 0:256], cos2[:, :])
                qk_r = qpool.tile([R, 256], BF16, tag="qkr")
                nc.vector.tensor_add(qk_r[:, :], qk_c[:, :], tmp[:, :])

                qT = qpool.tile([FPC, R], BF16, tag="qT")
                transpose_128(qk_r[:, 0:128], qT[:, :], bf=True)
                kTn = qpool.tile([FPC, R], BF16, tag="kTn")
                transpose_128(qk_r[:, 128:256], kTn[:, :], bf=True)

                # new-token V with ones columns, bf16: [v_h0|1|v_h1|1]
                vn = qpool.tile([128, 130], BF16, tag="vn")
                nc.vector.tensor_copy(vn[:, 0:Dh], qkv[:, 256:256 + Dh])
                nc.gpsimd.memset(vn[:, Dh:Dh + 1], 1.0)
                nc.vector.tensor_copy(vn[:, Dh + 1:2 * Dh + 1], qkv[:, 256 + Dh:256 + 2 * Dh])
                nc.gpsimd.memset(vn[:, 2 * Dh + 1:2 * Dh + 2], 1.0)

                O = qpool.tile([R, FPC], F32, tag="O")
                for hh in range(HPC):
                    hs = slice(Dh * hh, Dh * (hh + 1))
                    A = apool.tile([R, S + R], BF16, tag="A")
                    # prefix scores -> exp, 4 chunks of 512
                    for j in range(4):
                        s_ps = pps.tile([R, 512], F32, tag="ps512")
                        nc.tensor.matmul(s_ps[:, :], qT[hs, :],
                                         kt_sb[hs, 512 * j:512 * (j + 1)],
                                         start=True, stop=True)
                        nc.scalar.activation(A[:, 512 * j:512 * (j + 1)], s_ps[:, :],
                                             mybir.ActivationFunctionType.Exp,
                                             scale=1.0 / np.sqrt(Dh))
                    # new-token scores (full [R, R], then block-diag mask)
                    sn_ps = ppt.tile([128, 128], F32, tag="tp")
                    nc.tensor.matmul(sn_ps[:, :], qT[hs, :], kTn[hs, :],
                                     start=True, stop=True)
                    en = qpool.tile([R, R], BF16, tag="expn")
                    nc.scalar.activation(en[:, :], sn_ps[:, :],
                                         mybir.ActivationFunctionType.Exp,
                                         scale=1.0 / np.sqrt(Dh))
                    nc.vector.tensor_mul(A[:, S:S + R], en[:, :], mask01[:, :])

                    # A^T tiles + A@V accumulation (ones-col gives softmax sum)
                    AT = apool.tile([128, NT * 128], BF16, tag="AT")
                    av_ps = ppa.tile([R, Dh + 1], F32, tag="av")
                    for t in range(NT):
                        transpose_128(A[:, 128 * t:128 * (t + 1)],
                                      AT[:, 128 * t:128 * (t + 1)], bf=True)
                        if t < NT_PRE:
                            rhs = v_sb[:, 130 * t + 65 * hh: 130 * t + 65 * hh + 65]
                        else:
                            rhs = vn[:, 65 * hh: 65 * hh + 65]
                        nc.tensor.matmul(av_ps[:, :],
                                         AT[:, 128 * t:128 * (t + 1)], rhs,
                                         start=(t == 0), stop=(t == NT - 1))
                    sums = spool.tile([R, 1], F32, tag="smsums")
                    nc.vector.reciprocal(sums[:, 0:1], av_ps[:, Dh:Dh + 1])
                    nc.vector.tensor_scalar(O[:, Dh * hh:Dh * (hh + 1)],
                                            av_ps[:, 0:Dh],
                                            sums[:, 0:1], None,
                                            op0=mybir.AluOpType.mult)

                OT = qpool.tile([FPC, R], RDT, tag="OT")
                transpose_128(O[:, :], OT[:, :])
                wo = wpool.tile([FPC, D], RDT, tag="wo")
                nc.sync.dma_start(wo[:, :], P[f'wo{bi}'][:, :])
                y_attn = qpool.tile([R, D], BF16, tag="y2")
                for j in range(2):
                    y_ps = pps.tile([R, 512], F32, tag="ps512")
                    nc.tensor.matmul(y_ps[:, :], OT[:, :],
                                     wo[:, 512 * j:512 * (j + 1)],
                                     start=True, stop=True)
                    nc.vector.tensor_copy(y_attn[:, 512 * j:512 * (j + 1)], y_ps[:, :])
                all_reduce_add(y_attn, tag=f"a{bi}")

            def mlp_block(l):
                h = layer_norm(f"m{l}")
                hT = transpose_hT(h)

                a_ps = pps.tile([R, DFC], F32, tag="ps512")
                for kt_i in range(8):
                    w = wpool.tile([128, DFC], RDT, tag="w1")
                    nc.sync.dma_start(w[:, :], P[f'w1_{l}'][kt_i])
                    nc.tensor.matmul(a_ps[:, :],
                                     hT[:, 128 * kt_i:128 * (kt_i + 1)],
                                     w[:, :],
                                     start=(kt_i == 0), stop=(kt_i == 7))
                b1 = wpool.tile([R, DFC], F32, tag="b1")
                nc.sync.dma_start(b1[:, :], P[f'b1_{l}'][:, :])
                ab = qpool.tile([R, DFC], F32, tag="ab")
                nc.vector.tensor_add(ab[:, :], a_ps[:, :], b1[:, :])
                ag = qpool.tile([R, DFC], F32, tag="ag")
                nc.scalar.activation(ag[:, :], ab[:, :],
                                     mybir.ActivationFunctionType.Gelu_apprx_tanh)
                aT = hpool.tile([128, DFC], RDT, tag="aT")
                for i in range(4):
                    transpose_128(ag[:, 128 * i:128 * (i + 1)],
                                  aT[:, 128 * i:128 * (i + 1)])

                y_ps = [pps.tile([R, 512], F32, tag="ps512", name=f"y2ps{j}")
                        for j in range(2)]
                for kt_i in range(4):
                    w = wpool.tile([128, D], RDT, tag="w2")
                    nc.sync.dma_start(w[:, :], P[f'w2_{l}'][kt_i])
                    for j in range(2):
                        nc.tensor.matmul(y_ps[j][:, :],
                                         aT[:, 128 * kt_i:128 * (kt_i + 1)],
                                         w[:, 512 * j:512 * (j + 1)],
                                         start=(kt_i == 0), stop=(kt_i == 3))
                b2 = wpool.tile([R, D], F32, tag="b2")
                nc.sync.dma_start(b2[:, :], P[f'b2_{l}'][:, :])
                y2 = qpool.tile([R, D], BF16, tag="y2")
                for j in range(2):
                    nc.vector.scalar_tensor_tensor(
                        y2[:, 512 * j:512 * (j + 1)], y_ps[j][:, :], 1.0,
                        b2[:, 512 * j:512 * (j + 1)],
                        op0=mybir.AluOpType.mult, op1=mybir.AluOpType.add)
                all_reduce_add(y2, tag=f"m{l}")

            for l in range(NL):
                attn_block(2 * l)
                attn_block(2 * l + 1)
                mlp_block(l)

            nc.sync.dma_start(out[:, :], x_sb[:, :])

    nc.compile()
    return nc


_cached_nc = None


def _get_nc():
    global _cached_nc
    if _cached_nc is None:
        _cached_nc = _build()
    return _cached_nc


def _run(inputs, trace=False):
    nc = _get_nc()
    in_maps = _prep_in_maps(inputs)
    res = run_bass_kernel_spmd(nc, in_maps, list(range(N_CORES)), trace=trace)
    y = res.results[0]['out'].reshape(B, L, D).astype(np.float32)
    return y, res


def kernel(**inputs):
    y, _ = _run(inputs, trace=False)
    return y


# revision 11
# speedup vs baseline: 1.3099x; 1.2763x over previous
"""PoET transformer-with-KV-prefix kernel for 8 Trainium2 NeuronCores.

Sharding: tensor-parallel over heads (2 heads/core) for attention and over
FFN columns (512/core) for the MLP.  Activations [B*L=128, D=1024] are
replicated; each block ends in an 8-core AllReduce (bf16) of the output
projection partial sums.  LayerNorm gains/biases are folded into the
following weight matrices host-side, so on-device LN is a pure normalize.

Attention scores are computed pre-transposed (K-tile stationary, q moving),
so exp() writes A^T directly and A@V needs no transposes.  The V tiles
carry a ones-column so the A@V accumulation also produces the softmax
denominator for free.
"""

import sys
import numpy as np

for _p in ("/opt/trn_rl_repo", "/root/.axon_site/_ro/trn_rl_repo"):
    if _p not in sys.path:
        sys.path.insert(0, _p)

import ml_dtypes
import concourse.bass as bass
import concourse.bacc as bacc
import concourse.mybir as mybir
from concourse.tile import TileContext
from concourse.bass_utils import run_bass_kernel_spmd

# Problem dims (hardcoded per spec)
NL, B, L, D, H, Dh, S, DF = 2, 8, 16, 1024, 16, 64, 2048, 4096
ROPE_BASE = 10000.0
LN_EPS = 1e-5

N_CORES = 8
R = B * L            # 128 token rows
HPC = H // N_CORES   # 2 heads per core
FPC = HPC * Dh       # 128 features per core
DFC = DF // N_CORES  # 512 ffn cols per core
NT_PRE = S // 128    # 16 prefix t-tiles
NT = NT_PRE + 1      # 17 t-tiles including the new-token tile

F32 = mybir.dt.float32
F32R = mybir.dt.float32r
BF16 = mybir.dt.bfloat16
NPBF = ml_dtypes.bfloat16
RG = [list(range(N_CORES))]

WARMUP_CC = True     # tiny AllGather at t=0 to absorb collective setup/skew
W_BF16 = True        # bf16 weights + bf16 activation-stationary matmuls
WDT = BF16 if W_BF16 else F32R
NPW = NPBF if W_BF16 else np.float32


# ---------------------------------------------------------------------------
# Host-side input prep: fold LN into weights, transpose KV, slice per core.
# ---------------------------------------------------------------------------

def _prep_in_maps(inp):
    f = lambda k: np.asarray(inp[k], dtype=np.float32)
    x = f('x').reshape(R, D)

    # rope tables (token-major): row r -> position S + r % L
    pos = (S + np.arange(R) % L).astype(np.float32)
    inv = ROPE_BASE ** (-np.arange(Dh // 2, dtype=np.float32) / (Dh // 2))
    ang = pos[:, None] * inv[None, :]              # [128, 32]
    cos32, sin32 = np.cos(ang), np.sin(ang)
    blk_cos = np.concatenate([cos32, cos32], 1)    # [128, 64]
    blk_ssin = np.concatenate([-sin32, sin32], 1)  # [128, 64]
    cos2 = np.tile(blk_cos, (1, 4)).astype(np.float32)    # [128, 256] (q_h0,q_h1,k_h0,k_h1)
    ssin2 = np.tile(blk_ssin, (1, 4)).astype(np.float32)

    # block-diagonal own-batch mask for the new-token scores (symmetric)
    mask01 = np.kron(np.eye(B, dtype=np.float32),
                     np.ones((L, L), np.float32)).astype(NPBF)

    shared = {'x': x, 'cos2': cos2, 'ssin2': ssin2, 'mask01': mask01}

    attn_specs = [(0, 'self'), (0, 'cross'), (1, 'self'), (1, 'cross')]
    per_core = [dict(shared) for _ in range(N_CORES)]

    for bi, (l, kind) in enumerate(attn_specs):
        g = f('ln1_g' if kind == 'self' else 'ln2_g')[l]
        be = f('ln1_b' if kind == 'self' else 'ln2_b')[l]
        Wq, Wk, Wv, Wo = (f(f'{kind}_W{m}')[l] for m in 'qkvo')
        k_mem = f(f'{kind}_k_mem')[l]   # [S, H, Dh]
        v_mem = f(f'{kind}_v_mem')[l]
        Wq_e, Wk_e, Wv_e = g[:, None] * Wq, g[:, None] * Wk, g[:, None] * Wv
        bq, bk, bv = be @ Wq, be @ Wk, be @ Wv   # [D]
        for c in range(N_CORES):
            cs = slice(c * FPC, (c + 1) * FPC)
            wqkv = np.concatenate([Wq_e[:, cs], Wk_e[:, cs], Wv_e[:, cs]], 1)  # [1024, 384]
            bqkv = np.concatenate([bq[cs], bk[cs], bv[cs]])                    # [384]
            m = per_core[c]
            m[f'wqkv{bi}'] = np.ascontiguousarray(wqkv.reshape(8, 128, 3 * FPC)).astype(NPW)
            m[f'bqkv{bi}'] = np.ascontiguousarray(np.tile(bqkv[None, :], (128, 1)))
            m[f'wo{bi}'] = np.ascontiguousarray(Wo[cs, :]).astype(NPW)         # [128, 1024]
            # K^T per head, feature-major: [128 (2h x 64), S], bf16
            kt = k_mem[:, 2 * c:2 * c + 2, :].transpose(1, 2, 0).reshape(FPC, S)
            m[f'kt{bi}'] = np.ascontiguousarray(kt).astype(NPBF)
            # V token-major tiles with ones-columns: [128, 16*130], bf16
            # col layout per t-tile: [v_h0 (64) | 1 | v_h1 (64) | 1]
            v = v_mem[:, 2 * c:2 * c + 2, :].reshape(NT_PRE, 128, 2, Dh)
            va = np.ones((128, NT_PRE, 2, Dh + 1), np.float32)
            va[:, :, :, :Dh] = v.transpose(1, 0, 2, 3)
            m[f'v{bi}'] = np.ascontiguousarray(va.reshape(128, NT_PRE * 130)).astype(NPBF)

    for l in range(NL):
        g3, b3 = f('ln3_g')[l], f('ln3_b')[l]
        W1, b1, W2, b2 = f('W1')[l], f('b1')[l], f('W2')[l], f('b2')[l]
        W1_e = g3[:, None] * W1
        b1_e = b1 + b3 @ W1
        for c in range(N_CORES):
            cs = slice(c * DFC, (c + 1) * DFC)
            m = per_core[c]
            m[f'w1_{l}'] = np.ascontiguousarray(W1_e[:, cs].reshape(8, 128, DFC)).astype(NPW)
            m[f'b1_{l}'] = np.ascontiguousarray(np.tile(b1_e[None, cs], (128, 1)))
            m[f'w2_{l}'] = np.ascontiguousarray(W2[cs, :].reshape(4, 128, D)).astype(NPW)
            m[f'b2_{l}'] = np.ascontiguousarray(np.tile(b2[None, :] / N_CORES, (128, 1)))
    return per_core


# ---------------------------------------------------------------------------
# Device program (SPMD; identical on all cores, per-core data via in_maps)
# ---------------------------------------------------------------------------

def _build():
    from concourse import masks

    nc = bacc.Bacc("TRN2", target_bir_lowering=False, debug=False,
                   num_devices=N_CORES)
    P = {}
    P['x'] = nc.declare_dram_parameter('x', [R, D], F32, isOutput=False)
    P['cos2'] = nc.declare_dram_parameter('cos2', [R, 256], F32, isOutput=False)
    P['ssin2'] = nc.declare_dram_parameter('ssin2', [R, 256], F32, isOutput=False)
    P['mask01'] = nc.declare_dram_parameter('mask01', [R, R], BF16, isOutput=False)
    for bi in range(4):
        P[f'wqkv{bi}'] = nc.declare_dram_parameter(f'wqkv{bi}', [8, 128, 3 * FPC], WDT, isOutput=False)
        P[f'bqkv{bi}'] = nc.declare_dram_parameter(f'bqkv{bi}', [R, 3 * FPC], F32, isOutput=False)
        P[f'wo{bi}'] = nc.declare_dram_parameter(f'wo{bi}', [FPC, D], WDT, isOutput=False)
        P[f'kt{bi}'] = nc.declare_dram_parameter(f'kt{bi}', [FPC, S], BF16, isOutput=False)
        P[f'v{bi}'] = nc.declare_dram_parameter(f'v{bi}', [128, NT_PRE * 130], BF16, isOutput=False)
    for l in range(NL):
        P[f'w1_{l}'] = nc.declare_dram_parameter(f'w1_{l}', [8, 128, DFC], WDT, isOutput=False)
        P[f'b1_{l}'] = nc.declare_dram_parameter(f'b1_{l}', [R, DFC], F32, isOutput=False)
        P[f'w2_{l}'] = nc.declare_dram_parameter(f'w2_{l}', [4, 128, D], WDT, isOutput=False)
        P[f'b2_{l}'] = nc.declare_dram_parameter(f'b2_{l}', [R, D], F32, isOutput=False)
    out = nc.declare_dram_parameter('out', [R, D], F32, isOutput=True)

    with TileContext(nc) as tc:
        with (
            tc.tile_pool(name="cpool", bufs=1) as cpool,
            tc.tile_pool(name="hpool", bufs=2) as hpool,
            tc.tile_pool(name="qpool", bufs=2) as qpool,
            tc.tile_pool(name="apool", bufs=2) as apool,
            tc.tile_pool(name="kvpool", bufs=2) as kvpool,
            tc.tile_pool(name="wpool", bufs=3) as wpool,
            tc.tile_pool(name="spool", bufs=4) as spool,
            tc.tile_pool(name="ppt", bufs=2, space="PSUM") as ppt,
            tc.tile_pool(name="pps", bufs=2, space="PSUM") as pps,
            tc.tile_pool(name="ppa", bufs=2, space="PSUM") as ppa,
            tc.tile_pool(name="dpool", bufs=2, space="DRAM") as dpool,
        ):
            identb = cpool.tile([128, 128], BF16, tag="identb")
            masks.make_identity(nc, identb[:, :])
            x_sb = cpool.tile([R, D], F32, tag="x")
            nc.sync.dma_start(x_sb[:, :], P['x'][:, :])
            cos2 = cpool.tile([R, 256], F32, tag="cos2")
            nc.sync.dma_start(cos2[:, :], P['cos2'][:, :])
            ssin2 = cpool.tile([R, 256], F32, tag="ssin2")
            nc.sync.dma_start(ssin2[:, :], P['ssin2'][:, :])
            mask01 = cpool.tile([R, R], BF16, tag="mask01")
            nc.sync.dma_start(mask01[:, :], P['mask01'][:, :])
            eps_t = cpool.tile([128, 1], F32, tag="eps")
            nc.gpsimd.memset(eps_t[:, :], LN_EPS)

            if WARMUP_CC:
                wu_in = dpool.tile([2, 16], F32, tag="wu_in")
                wu_out = dpool.tile([16, 16], F32, tag="wu_out")
                nc.gpsimd.dma_start(wu_in[:], P['x'][0:2, 0:16])
                nc.gpsimd.collective_compute(
                    "AllGather", mybir.AluOpType.bypass, replica_groups=RG,
                    ins=[wu_in.opt()], outs=[wu_out.opt()])

            def layer_norm(tag):
                """x_sb -> hb [R, D] bf16 normalized (gains folded away)."""
                sums = spool.tile([R, 8], F32, tag="lnsums")
                nc.vector.tensor_reduce(sums[:, 0:1], x_sb[:, :],
                                        axis=mybir.AxisListType.X, op=mybir.AluOpType.add)
                sq = hpool.tile([R, D], F32, tag="sq")
                nc.scalar.activation(sq[:, :], x_sb[:, :],
                                     mybir.ActivationFunctionType.Square,
                                     accum_out=sums[:, 1:2])             # sum(x^2)
                nc.vector.tensor_scalar(sums[:, 2:3], sums[:, 0:1], 1.0 / D,
                                        None, op0=mybir.AluOpType.mult)  # mean
                nc.vector.tensor_tensor(sums[:, 3:4], sums[:, 2:3], sums[:, 2:3],
                                        op=mybir.AluOpType.mult)         # mean^2
                nc.vector.tensor_scalar(sums[:, 4:5], sums[:, 3:4], -1.0,
                                        eps_t[:, 0:1], op0=mybir.AluOpType.mult,
                                        op1=mybir.AluOpType.add)         # eps - mean^2
                nc.scalar.activation(sums[:, 5:6], sums[:, 1:2],
                                     mybir.ActivationFunctionType.Sqrt,
                                     scale=1.0 / D, bias=sums[:, 4:5])   # sqrt(var+eps)
                nc.vector.reciprocal(sums[:, 6:7], sums[:, 5:6])         # rstd
                h = hpool.tile([R, D], WDT, tag="h")
                nc.vector.tensor_scalar(h[:, :], x_sb[:, :], sums[:, 2:3], sums[:, 6:7],
                                        op0=mybir.AluOpType.subtract,
                                        op1=mybir.AluOpType.mult)
                return h

            def transpose_128(src_ap, dst_ap):
                """PE-transpose one bf16 [128, <=128] slice into SBUF dst."""
                np_, nf = src_ap.shape[0], src_ap.shape[1]
                p = ppt.tile([128, 128], BF16, tag="tpb")
                nc.tensor.transpose(p[:nf, :np_], src_ap, identb[:np_, :np_])
                nc.vector.tensor_copy(dst_ap, p[:nf, :np_])

            def transpose_hT(h, n=8):
                hT = hpool.tile([R, D], WDT, tag="hT")
                for i in range(n):
                    transpose_128(h[:, 128 * i:128 * (i + 1)],
                                  hT[:, 128 * i:128 * (i + 1)])
                return hT

            def all_reduce_add(y_sb, tag=""):
                """AllReduce (bf16) the [R, D] partial and add into x_sb."""
                cin = dpool.tile([R, D], BF16, tag="cc_in")
                cout = dpool.tile([R, D], BF16, tag="cc_out")
                nc.gpsimd.dma_start(cin[:, :], y_sb[:, :])
                nc.gpsimd.collective_compute(
                    "AllReduce", mybir.AluOpType.add, replica_groups=RG,
                    ins=[cin.opt()], outs=[cout.opt()])
                y = hpool.tile([R, D], BF16, tag="yred")
                nc.sync.dma_start(y[:, :], cout[:, :])
                nc.vector.tensor_add(x_sb[:, :], x_sb[:, :], y[:, :])

            def attn_block(bi):
                # prefix K^T and V (prefetchable, no deps)
                kt_sb = kvpool.tile([FPC, S], BF16, tag="kt")
                nc.sync.dma_start(kt_sb[:, :], P[f'kt{bi}'][:, :])
                v_sb = kvpool.tile([128, NT_PRE * 130], BF16, tag="v")
                nc.sync.dma_start(v_sb[:, :], P[f'v{bi}'][:, :])

                h = layer_norm(f"a{bi}")
                hT = transpose_hT(h)

                # qkv = h @ Wqkv_c + bqkv   [R, 384] (token-major)
                qkv_ps = pps.tile([R, 3 * FPC], F32, tag="ps512")
                for kt_i in range(8):
                    w = wpool.tile([128, 3 * FPC], WDT, tag="wqkv")
                    nc.sync.dma_start(w[:, :], P[f'wqkv{bi}'][kt_i])
                    nc.tensor.matmul(qkv_ps[:, :],
                                     hT[:, 128 * kt_i:128 * (kt_i + 1)],
                                     w[:, :],
                                     start=(kt_i == 0), stop=(kt_i == 7))
                bq = wpool.tile([R, 3 * FPC], F32, tag="bqkv")
                nc.sync.dma_start(bq[:, :], P[f'bqkv{bi}'][:, :])
                qkv = qpool.tile([R, 3 * FPC], F32, tag="qkv")
                nc.vector.tensor_add(qkv[:, :], qkv_ps[:, :], bq[:, :])

                # rope on q|k region [R, 256] -> bf16
                tmp = qpool.tile([R, 256], F32, tag="ropetmp")
                for blk in range(4):
                    a0, a1, a2 = 64 * blk, 64 * blk + 32, 64 * blk + 64
                    nc.vector.tensor_mul(tmp[:, a0:a1], qkv[:, a1:a2], ssin2[:, a0:a1])
                    nc.vector.tensor_mul(tmp[:, a1:a2], qkv[:, a0:a1], ssin2[:, a1:a2])
                qk_c = qpool.tile([R, 256], F32, tag="qkc")
                nc.vector.tensor_mul(qk_c[:, :], qkv[:, 0:256], cos2[:, :])
                qk_r = qpool.tile([R, 256], BF16, tag="qkr")
                nc.vector.tensor_add(qk_r[:, :], qk_c[:, :], tmp[:, :])

                qT = qpool.tile([FPC, R], BF16, tag="qT")
                transpose_128(qk_r[:, 0:128], qT[:, :])
                kTn = qpool.tile([FPC, R], BF16, tag="kTn")
                transpose_128(qk_r[:, 128:256], kTn[:, :])

                # new-token V with ones columns, bf16: [v_h0|1|v_h1|1]
                vn = qpool.tile([128, 130], BF16, tag="vn")
                nc.vector.tensor_copy(vn[:, 0:Dh], qkv[:, 256:256 + Dh])
                nc.gpsimd.memset(vn[:, Dh:Dh + 1], 1.0)
                nc.vector.tensor_copy(vn[:, Dh + 1:2 * Dh + 1], qkv[:, 256 + Dh:256 + 2 * Dh])
                nc.gpsimd.memset(vn[:, 2 * Dh + 1:2 * Dh + 2], 1.0)

                O = qpool.tile([R, FPC], BF16, tag="O")
                inv_sqrt_d = 1.0 / np.sqrt(Dh)
                for hh in range(HPC):
                    hs = slice(Dh * hh, Dh * (hh + 1))
                    # A^T computed directly: scores pre-transposed
                    # (K-tile stationary, q moving), exp PSUM->SBUF
                    AT = apool.tile([128, NT * 128], BF16, tag="AT")
                    av_ps = ppa.tile([R, Dh + 1], F32, tag="av")
                    for j in range(4):
                        s_ps = pps.tile([R, 512], F32, tag="ps512")
                        for tt in range(4):
                            t = 4 * j + tt
                            nc.tensor.matmul(s_ps[:, 128 * tt:128 * (tt + 1)],
                                             kt_sb[hs, 128 * t:128 * (t + 1)],
                                             qT[hs, :], start=True, stop=True)
                        nc.scalar.activation(AT[:, 512 * j:512 * (j + 1)], s_ps[:, :],
                                             mybir.ActivationFunctionType.Exp,
                                             scale=inv_sqrt_d)
                        for tt in range(4):
                            t = 4 * j + tt
                            nc.tensor.matmul(
                                av_ps[:, :],
                                AT[:, 128 * t:128 * (t + 1)],
                                v_sb[:, 130 * t + 65 * hh: 130 * t + 65 * hh + 65],
                                start=(t == 0), stop=False)
                    # new-token scores (transposed, [new_tok, row]), masked
                    sn_ps = ppt.tile([128, 128], F32, tag="snp")
                    nc.tensor.matmul(sn_ps[:, :], kTn[hs, :], qT[hs, :],
                                     start=True, stop=True)
                    en = qpool.tile([R, R], BF16, tag="expn")
                    nc.scalar.activation(en[:, :], sn_ps[:, :],
                                         mybir.ActivationFunctionType.Exp,
                                         scale=inv_sqrt_d)
                    nc.vector.tensor_mul(AT[:, S:S + R], en[:, :], mask01[:, :])
                    nc.tensor.matmul(av_ps[:, :], AT[:, S:S + R],
                                     vn[:, 65 * hh: 65 * hh + 65],
                                     start=False, stop=True)
                    sums = spool.tile([R, 1], F32, tag="smsums")
                    nc.vector.reciprocal(sums[:, 0:1], av_ps[:, Dh:Dh + 1])
                    nc.vector.tensor_scalar(O[:, Dh * hh:Dh * (hh + 1)],
                                            av_ps[:, 0:Dh],
                                            sums[:, 0:1], None,
                                            op0=mybir.AluOpType.mult)

                OT = qpool.tile([FPC, R], WDT, tag="OT")
                transpose_128(O[:, :], OT[:, :])
                wo = wpool.tile([FPC, D], WDT, tag="wo")
                nc.sync.dma_start(wo[:, :], P[f'wo{bi}'][:, :])
                y_attn = qpool.tile([R, D], BF16, tag="y2")
                for j in range(2):
                    y_ps = pps.tile([R, 512], F32, tag="ps512")
                    nc.tensor.matmul(y_ps[:, :], OT[:, :],
                                     wo[:, 512 * j:512 * (j + 1)],
                                     start=True, stop=True)
                    nc.vector.tensor_copy(y_attn[:, 512 * j:512 * (j + 1)], y_ps[:, :])
                all_reduce_add(y_attn, tag=f"a{bi}")

            def mlp_block(l):
                h = layer_norm(f"m{l}")
                hT = transpose_hT(h)

                a_ps = pps.tile([R, DFC], F32, tag="ps512")
                for kt_i in range(8):
                    w = wpool.tile([128, DFC], WDT, tag="w1")
                    nc.sync.dma_start(w[:, :], P[f'w1_{l}'][kt_i])
                    nc.tensor.matmul(a_ps[:, :],
                                     hT[:, 128 * kt_i:128 * (kt_i + 1)],
                                     w[:, :],
                                     start=(kt_i == 0), stop=(kt_i == 7))
                b1 = wpool.tile([R, DFC], F32, tag="b1")
                nc.sync.dma_start(b1[:, :], P[f'b1_{l}'][:, :])
                ab = qpool.tile([R, DFC], F32, tag="ab")
                nc.vector.tensor_add(ab[:, :], a_ps[:, :], b1[:, :])
                ag = qpool.tile([R, DFC], WDT, tag="ag")
                nc.scalar.activation(ag[:, :], ab[:, :],
                                     mybir.ActivationFunctionType.Gelu_apprx_tanh)
                aT = hpool.tile([128, DFC], WDT, tag="aT")
                for i in range(4):
                    transpose_128(ag[:, 128 * i:128 * (i + 1)],
                                  aT[:, 128 * i:128 * (i + 1)])

                y_ps = [pps.tile([R, 512], F32, tag="ps512", name=f"y2ps{j}")
                        for j in range(2)]
                for kt_i in range(4):
                    w = wpool.tile([128, D], WDT, tag="w2")
                    nc.sync.dma_start(w[:, :], P[f'w2_{l}'][kt_i])
                    for j in range(2):
                        nc.tensor.matmul(y_ps[j][:, :],
                                         aT[:, 128 * kt_i:128 * (kt_i + 1)],
                                         w[:, 512 * j:512 * (j + 1)],
                                         start=(kt_i == 0), stop=(kt_i == 3))
                b2 = wpool.tile([R, D], F32, tag="b2")
                nc.sync.dma_start(b2[:, :], P[f'b2_{l}'][:, :])
                y2 = qpool.tile([R, D], BF16, tag="y2")
                for j in range(2):
                    nc.vector.scalar_tensor_tensor(
                        y2[:, 512 * j:512 * (j + 1)], y_ps[j][:, :], 1.0,
                        b2[:, 512 * j:512 * (j + 1)],
                        op0=mybir.AluOpType.mult, op1=mybir.AluOpType.add)
                all_reduce_add(y2, tag=f"m{l}")

            for l in range(NL):
                attn_block(2 * l)
                attn_block(2 * l + 1)
                mlp_block(l)

            nc.sync.dma_start(out[:, :], x_sb[:, :])

    nc.compile()
    return nc


_cached_nc = None


def _get_nc():
    global _cached_nc
    if _cached_nc is None:
        _cached_nc = _build()
    return _cached_nc


def _run(inputs, trace=False):
    nc = _get_nc()
    in_maps = _prep_in_maps(inputs)
    res = run_bass_kernel_spmd(nc, in_maps, list(range(N_CORES)), trace=trace)
    y = res.results[0]['out'].reshape(B, L, D).astype(np.float32)
    return y, res


def kernel(**inputs):
    y, _ = _run(inputs, trace=False)
    return y


# revision 14
# speedup vs baseline: 1.3403x; 1.0232x over previous
"""PoET transformer-with-KV-prefix kernel for 8 Trainium2 NeuronCores.

Sharding: tensor-parallel over heads (2 heads/core) for attention and over
FFN columns (512/core) for the MLP.  Activations [B*L=128, D=1024] are
replicated; each block ends in an 8-core AllReduce (bf16) of the output
projection partial sums.  LayerNorm gains/biases are folded into the
following weight matrices host-side, so on-device LN is a pure normalize.

Attention scores are computed pre-transposed (K-tile stationary, q moving),
so exp() writes A^T directly and A@V needs no transposes.  The V tiles
carry a ones-column so the A@V accumulation also produces the softmax
denominator for free.
"""

import sys
import numpy as np

for _p in ("/opt/trn_rl_repo", "/root/.axon_site/_ro/trn_rl_repo"):
    if _p not in sys.path:
        sys.path.insert(0, _p)

import ml_dtypes
import concourse.bass as bass
import concourse.bacc as bacc
import concourse.mybir as mybir
from concourse.tile import TileContext
from concourse.bass_utils import run_bass_kernel_spmd

# Problem dims (hardcoded per spec)
NL, B, L, D, H, Dh, S, DF = 2, 8, 16, 1024, 16, 64, 2048, 4096
ROPE_BASE = 10000.0
LN_EPS = 1e-5

N_CORES = 8
R = B * L            # 128 token rows
HPC = H // N_CORES   # 2 heads per core
FPC = HPC * Dh       # 128 features per core
DFC = DF // N_CORES  # 512 ffn cols per core
NT_PRE = S // 128    # 16 prefix t-tiles
NT = NT_PRE + 1      # 17 t-tiles including the new-token tile

F32 = mybir.dt.float32
F32R = mybir.dt.float32r
BF16 = mybir.dt.bfloat16
NPBF = ml_dtypes.bfloat16
RG = [list(range(N_CORES))]

WARMUP_CC = False    # tiny AllGather at t=0 to absorb collective setup/skew
W_BF16 = True        # bf16 weights + bf16 activation-stationary matmuls
WDT = BF16 if W_BF16 else F32R
NPW = NPBF if W_BF16 else np.float32


# ---------------------------------------------------------------------------
# Host-side input prep: fold LN into weights, transpose KV, slice per core.
# ---------------------------------------------------------------------------

def _prep_in_maps(inp):
    f = lambda k: np.asarray(inp[k], dtype=np.float32)
    x = f('x').reshape(R, D)

    # rope tables (token-major): row r -> position S + r % L
    pos = (S + np.arange(R) % L).astype(np.float32)
    inv = ROPE_BASE ** (-np.arange(Dh // 2, dtype=np.float32) / (Dh // 2))
    ang = pos[:, None] * inv[None, :]              # [128, 32]
    cos32, sin32 = np.cos(ang), np.sin(ang)
    blk_cos = np.concatenate([cos32, cos32], 1)    # [128, 64]
    blk_ssin = np.concatenate([-sin32, sin32], 1)  # [128, 64]
    cos2 = np.tile(blk_cos, (1, 4)).astype(np.float32)    # [128, 256] (q_h0,q_h1,k_h0,k_h1)
    ssin2 = np.tile(blk_ssin, (1, 4)).astype(np.float32)

    # block-diagonal own-batch mask for the new-token scores (symmetric)
    mask01 = np.kron(np.eye(B, dtype=np.float32),
                     np.ones((L, L), np.float32)).astype(NPBF)

    shared = {'x': x, 'cos2': cos2, 'ssin2': ssin2, 'mask01': mask01}

    attn_specs = [(0, 'self'), (0, 'cross'), (1, 'self'), (1, 'cross')]
    per_core = [dict(shared) for _ in range(N_CORES)]

    for bi, (l, kind) in enumerate(attn_specs):
        g = f('ln1_g' if kind == 'self' else 'ln2_g')[l]
        be = f('ln1_b' if kind == 'self' else 'ln2_b')[l]
        Wq, Wk, Wv, Wo = (f(f'{kind}_W{m}')[l] for m in 'qkvo')
        k_mem = f(f'{kind}_k_mem')[l]   # [S, H, Dh]
        v_mem = f(f'{kind}_v_mem')[l]
        Wq_e, Wk_e, Wv_e = g[:, None] * Wq, g[:, None] * Wk, g[:, None] * Wv
        bq, bk, bv = be @ Wq, be @ Wk, be @ Wv   # [D]
        for c in range(N_CORES):
            cs = slice(c * FPC, (c + 1) * FPC)
            wqkv = np.concatenate([Wq_e[:, cs], Wk_e[:, cs], Wv_e[:, cs]], 1)  # [1024, 384]
            bqkv = np.concatenate([bq[cs], bk[cs], bv[cs]])                    # [384]
            m = per_core[c]
            m[f'wqkv{bi}'] = np.ascontiguousarray(wqkv.reshape(8, 128, 3 * FPC)).astype(NPW)
            m[f'bqkv{bi}'] = np.ascontiguousarray(np.tile(bqkv[None, :], (128, 1)))
            m[f'wo{bi}'] = np.ascontiguousarray(Wo[cs, :]).astype(NPW)         # [128, 1024]
            # K^T per head, feature-major: [128 (2h x 64), S], bf16
            kt = k_mem[:, 2 * c:2 * c + 2, :].transpose(1, 2, 0).reshape(FPC, S)
            m[f'kt{bi}'] = np.ascontiguousarray(kt).astype(NPBF)
            # V token-major tiles with ones-columns: [128, 16*130], bf16
            # col layout per t-tile: [v_h0 (64) | 1 | v_h1 (64) | 1]
            v = v_mem[:, 2 * c:2 * c + 2, :].reshape(NT_PRE, 128, 2, Dh)
            va = np.ones((128, NT_PRE, 2, Dh + 1), np.float32)
            va[:, :, :, :Dh] = v.transpose(1, 0, 2, 3)
            m[f'v{bi}'] = np.ascontiguousarray(va.reshape(128, NT_PRE * 130)).astype(NPBF)

    for l in range(NL):
        g3, b3 = f('ln3_g')[l], f('ln3_b')[l]
        W1, b1, W2, b2 = f('W1')[l], f('b1')[l], f('W2')[l], f('b2')[l]
        W1_e = g3[:, None] * W1
        b1_e = b1 + b3 @ W1
        for c in range(N_CORES):
            cs = slice(c * DFC, (c + 1) * DFC)
            m = per_core[c]
            m[f'w1_{l}'] = np.ascontiguousarray(W1_e[:, cs].reshape(8, 128, DFC)).astype(NPW)
            m[f'b1_{l}'] = np.ascontiguousarray(np.tile(b1_e[None, cs], (128, 1)))
            m[f'w2_{l}'] = np.ascontiguousarray(W2[cs, :].reshape(4, 128, D)).astype(NPW)
            m[f'b2_{l}'] = np.ascontiguousarray(np.tile(b2[None, :] / N_CORES, (128, 1)))
    return per_core


# ---------------------------------------------------------------------------
# Device program (SPMD; identical on all cores, per-core data via in_maps)
# ---------------------------------------------------------------------------

def _build():
    from concourse import masks

    nc = bacc.Bacc("TRN2", target_bir_lowering=False, debug=False,
                   num_devices=N_CORES)
    P = {}
    P['x'] = nc.declare_dram_parameter('x', [R, D], F32, isOutput=False)
    P['cos2'] = nc.declare_dram_parameter('cos2', [R, 256], F32, isOutput=False)
    P['ssin2'] = nc.declare_dram_parameter('ssin2', [R, 256], F32, isOutput=False)
    P['mask01'] = nc.declare_dram_parameter('mask01', [R, R], BF16, isOutput=False)
    for bi in range(4):
        P[f'wqkv{bi}'] = nc.declare_dram_parameter(f'wqkv{bi}', [8, 128, 3 * FPC], WDT, isOutput=False)
        P[f'bqkv{bi}'] = nc.declare_dram_parameter(f'bqkv{bi}', [R, 3 * FPC], F32, isOutput=False)
        P[f'wo{bi}'] = nc.declare_dram_parameter(f'wo{bi}', [FPC, D], WDT, isOutput=False)
        P[f'kt{bi}'] = nc.declare_dram_parameter(f'kt{bi}', [FPC, S], BF16, isOutput=False)
        P[f'v{bi}'] = nc.declare_dram_parameter(f'v{bi}', [128, NT_PRE * 130], BF16, isOutput=False)
    for l in range(NL):
        P[f'w1_{l}'] = nc.declare_dram_parameter(f'w1_{l}', [8, 128, DFC], WDT, isOutput=False)
        P[f'b1_{l}'] = nc.declare_dram_parameter(f'b1_{l}', [R, DFC], F32, isOutput=False)
        P[f'w2_{l}'] = nc.declare_dram_parameter(f'w2_{l}', [4, 128, D], WDT, isOutput=False)
        P[f'b2_{l}'] = nc.declare_dram_parameter(f'b2_{l}', [R, D], F32, isOutput=False)
    out = nc.declare_dram_parameter('out', [R, D], F32, isOutput=True)

    with TileContext(nc) as tc:
        with (
            tc.tile_pool(name="cpool", bufs=1) as cpool,
            tc.tile_pool(name="hpool", bufs=2) as hpool,
            tc.tile_pool(name="qpool", bufs=2) as qpool,
            tc.tile_pool(name="apool", bufs=2) as apool,
            tc.tile_pool(name="kvpool", bufs=2) as kvpool,
            tc.tile_pool(name="wpool", bufs=3) as wpool,
            tc.tile_pool(name="spool", bufs=4) as spool,
            tc.tile_pool(name="ppt", bufs=2, space="PSUM") as ppt,
            tc.tile_pool(name="pps", bufs=4, space="PSUM") as pps,
            tc.tile_pool(name="ppa", bufs=2, space="PSUM") as ppa,
            tc.tile_pool(name="dpool", bufs=2, space="DRAM") as dpool,
        ):
            identb = cpool.tile([128, 128], BF16, tag="identb")
            masks.make_identity(nc, identb[:, :])
            x_sb = cpool.tile([R, D], F32, tag="x")
            nc.sync.dma_start(x_sb[:, :], P['x'][:, :])
            cos2 = cpool.tile([R, 256], F32, tag="cos2")
            nc.sync.dma_start(cos2[:, :], P['cos2'][:, :])
            ssin2 = cpool.tile([R, 256], F32, tag="ssin2")
            nc.sync.dma_start(ssin2[:, :], P['ssin2'][:, :])
            mask01 = cpool.tile([R, R], BF16, tag="mask01")
            nc.sync.dma_start(mask01[:, :], P['mask01'][:, :])
            eps_t = cpool.tile([128, 1], F32, tag="eps")
            nc.gpsimd.memset(eps_t[:, :], LN_EPS)

            if WARMUP_CC:
                wu_in = dpool.tile([2, 16], F32, tag="wu_in")
                wu_out = dpool.tile([16, 16], F32, tag="wu_out")
                nc.gpsimd.dma_start(wu_in[:], P['x'][0:2, 0:16])
                nc.gpsimd.collective_compute(
                    "AllGather", mybir.AluOpType.bypass, replica_groups=RG,
                    ins=[wu_in.opt()], outs=[wu_out.opt()])

            def layer_norm(tag):
                """x_sb -> hb [R, D] bf16 normalized (gains folded away)."""
                sums = spool.tile([R, 8], F32, tag="lnsums")
                nc.vector.tensor_reduce(sums[:, 0:1], x_sb[:, :],
                                        axis=mybir.AxisListType.X, op=mybir.AluOpType.add)
                sq = hpool.tile([R, D], F32, tag="sq")
                nc.scalar.activation(sq[:, :], x_sb[:, :],
                                     mybir.ActivationFunctionType.Square,
                                     accum_out=sums[:, 1:2])             # sum(x^2)
                nc.vector.tensor_scalar(sums[:, 2:3], sums[:, 0:1], 1.0 / D,
                                        None, op0=mybir.AluOpType.mult)  # mean
                nc.vector.tensor_tensor(sums[:, 3:4], sums[:, 2:3], sums[:, 2:3],
                                        op=mybir.AluOpType.mult)         # mean^2
                nc.vector.tensor_scalar(sums[:, 4:5], sums[:, 3:4], -1.0,
                                        eps_t[:, 0:1], op0=mybir.AluOpType.mult,
                                        op1=mybir.AluOpType.add)         # eps - mean^2
                nc.scalar.activation(sums[:, 5:6], sums[:, 1:2],
                                     mybir.ActivationFunctionType.Sqrt,
                                     scale=1.0 / D, bias=sums[:, 4:5])   # sqrt(var+eps)
                nc.vector.reciprocal(sums[:, 6:7], sums[:, 5:6])         # rstd
                h = hpool.tile([R, D], WDT, tag="h")
                nc.vector.tensor_scalar(h[:, :], x_sb[:, :], sums[:, 2:3], sums[:, 6:7],
                                        op0=mybir.AluOpType.subtract,
                                        op1=mybir.AluOpType.mult)
                return h

            def transpose_128(src_ap, dst_ap):
                """PE-transpose one bf16 [128, <=128] slice into SBUF dst."""
                np_, nf = src_ap.shape[0], src_ap.shape[1]
                p = ppt.tile([128, 128], BF16, tag="tpb")
                nc.tensor.transpose(p[:nf, :np_], src_ap, identb[:np_, :np_])
                nc.vector.tensor_copy(dst_ap, p[:nf, :np_])

            def transpose_hT(h, n=8):
                hT = hpool.tile([R, D], WDT, tag="hT")
                for i in range(n):
                    transpose_128(h[:, 128 * i:128 * (i + 1)],
                                  hT[:, 128 * i:128 * (i + 1)])
                return hT

            def all_reduce_add(y_sb, tag=""):
                """AllReduce (bf16) the [R, D] partial and add into x_sb."""
                cin = dpool.tile([R, D], BF16, tag="cc_in")
                cout = dpool.tile([R, D], BF16, tag="cc_out")
                nc.gpsimd.dma_start(cin[:, :], y_sb[:, :])
                nc.gpsimd.collective_compute(
                    "AllReduce", mybir.AluOpType.add, replica_groups=RG,
                    ins=[cin.opt()], outs=[cout.opt()])
                y = hpool.tile([R, D], BF16, tag="yred")
                nc.sync.dma_start(y[:, :], cout[:, :])
                nc.vector.tensor_add(x_sb[:, :], x_sb[:, :], y[:, :])

            def attn_block(bi):
                # prefix K^T and V (prefetchable, no deps)
                kt_sb = kvpool.tile([FPC, S], BF16, tag="kt")
                nc.sync.dma_start(kt_sb[:, :], P[f'kt{bi}'][:, :])
                v_sb = kvpool.tile([128, NT_PRE * 130], BF16, tag="v")
                nc.sync.dma_start(v_sb[:, :], P[f'v{bi}'][:, :])

                h = layer_norm(f"a{bi}")
                hT = transpose_hT(h)

                # qkv = h @ Wqkv_c + bqkv   [R, 384] (token-major)
                qkv_ps = pps.tile([R, 3 * FPC], F32, tag="ps512")
                for kt_i in range(8):
                    w = wpool.tile([128, 3 * FPC], WDT, tag="wqkv")
                    nc.sync.dma_start(w[:, :], P[f'wqkv{bi}'][kt_i])
                    nc.tensor.matmul(qkv_ps[:, :],
                                     hT[:, 128 * kt_i:128 * (kt_i + 1)],
                                     w[:, :],
                                     start=(kt_i == 0), stop=(kt_i == 7))
                bq = wpool.tile([R, 3 * FPC], F32, tag="bqkv")
                nc.sync.dma_start(bq[:, :], P[f'bqkv{bi}'][:, :])
                qkv = qpool.tile([R, 3 * FPC], F32, tag="qkv")
                nc.vector.tensor_add(qkv[:, :], qkv_ps[:, :], bq[:, :])

                # rope on q|k region [R, 256] -> bf16
                tmp = qpool.tile([R, 256], F32, tag="ropetmp")
                for blk in range(4):
                    a0, a1, a2 = 64 * blk, 64 * blk + 32, 64 * blk + 64
                    nc.vector.tensor_mul(tmp[:, a0:a1], qkv[:, a1:a2], ssin2[:, a0:a1])
                    nc.vector.tensor_mul(tmp[:, a1:a2], qkv[:, a0:a1], ssin2[:, a1:a2])
                qk_c = qpool.tile([R, 256], F32, tag="qkc")
                nc.vector.tensor_mul(qk_c[:, :], qkv[:, 0:256], cos2[:, :])
                qk_r = qpool.tile([R, 256], BF16, tag="qkr")
                nc.vector.tensor_add(qk_r[:, :], qk_c[:, :], tmp[:, :])

                qT = qpool.tile([FPC, R], BF16, tag="qT")
                transpose_128(qk_r[:, 0:128], qT[:, :])
                kTn = qpool.tile([FPC, R], BF16, tag="kTn")
                transpose_128(qk_r[:, 128:256], kTn[:, :])

                # new-token V with ones columns, bf16: [v_h0|1|v_h1|1]
                vn = qpool.tile([128, 130], BF16, tag="vn")
                nc.vector.tensor_copy(vn[:, 0:Dh], qkv[:, 256:256 + Dh])
                nc.gpsimd.memset(vn[:, Dh:Dh + 1], 1.0)
                nc.vector.tensor_copy(vn[:, Dh + 1:2 * Dh + 1], qkv[:, 256 + Dh:256 + 2 * Dh])
                nc.gpsimd.memset(vn[:, 2 * Dh + 1:2 * Dh + 2], 1.0)

                O = qpool.tile([R, FPC], BF16, tag="O")
                inv_sqrt_d = 1.0 / np.sqrt(Dh)
                # Both heads interleaved: scores pre-transposed (K-tile
                # stationary, q moving) so exp writes A^T directly; A@V
                # accumulates with the ones-column giving the softmax sum.
                hslices = [slice(Dh * hh, Dh * (hh + 1)) for hh in range(HPC)]
                ATs = [apool.tile([128, NT * 128], BF16, tag="AT", name=f"AT{bi}_{hh}")
                       for hh in range(HPC)]
                avs = [ppa.tile([R, Dh + 1], F32, tag="av", name=f"av{bi}_{hh}")
                       for hh in range(HPC)]
                for j in range(4):
                    sps = []
                    for hh in range(HPC):
                        s_ps = pps.tile([R, 512], F32, tag="ps512", name=f"sps{hh}")
                        for tt in range(4):
                            t = 4 * j + tt
                            nc.tensor.matmul(s_ps[:, 128 * tt:128 * (tt + 1)],
                                             kt_sb[hslices[hh], 128 * t:128 * (t + 1)],
                                             qT[hslices[hh], :], start=True, stop=True)
                        sps.append(s_ps)
                    for hh in range(HPC):
                        nc.scalar.activation(ATs[hh][:, 512 * j:512 * (j + 1)],
                                             sps[hh][:, :],
                                             mybir.ActivationFunctionType.Exp,
                                             scale=inv_sqrt_d)
                    for hh in range(HPC):
                        for tt in range(4):
                            t = 4 * j + tt
                            nc.tensor.matmul(
                                avs[hh][:, :],
                                ATs[hh][:, 128 * t:128 * (t + 1)],
                                v_sb[:, 130 * t + 65 * hh: 130 * t + 65 * hh + 65],
                                start=(t == 0), stop=False)
                # new-token scores (transposed, [new_tok, row]), masked
                for hh in range(HPC):
                    sn_ps = ppt.tile([128, 128], F32, tag="tpb", name=f"snp{hh}")
                    nc.tensor.matmul(sn_ps[:, :], kTn[hslices[hh], :],
                                     qT[hslices[hh], :], start=True, stop=True)
                    en = qpool.tile([R, R], BF16, tag="expn", name=f"en{hh}")
                    nc.scalar.activation(en[:, :], sn_ps[:, :],
                                         mybir.ActivationFunctionType.Exp,
                                         scale=inv_sqrt_d)
                    nc.vector.tensor_mul(ATs[hh][:, S:S + R], en[:, :], mask01[:, :])
                    nc.tensor.matmul(avs[hh][:, :], ATs[hh][:, S:S + R],
                                     vn[:, 65 * hh: 65 * hh + 65],
                                     start=False, stop=True)
                sums = spool.tile([R, 2], F32, tag="smsums")
                for hh in range(HPC):
                    nc.vector.reciprocal(sums[:, hh:hh + 1], avs[hh][:, Dh:Dh + 1])
                    nc.vector.tensor_scalar(O[:, Dh * hh:Dh * (hh + 1)],
                                            avs[hh][:, 0:Dh],
                                            sums[:, hh:hh + 1], None,
                                            op0=mybir.AluOpType.mult)

                OT = qpool.tile([FPC, R], WDT, tag="OT")
                transpose_128(O[:, :], OT[:, :])
                wo = wpool.tile([FPC, D], WDT, tag="wo")
                nc.sync.dma_start(wo[:, :], P[f'wo{bi}'][:, :])
                y_attn = qpool.tile([R, D], BF16, tag="y2")
                for j in range(2):
                    y_ps = pps.tile([R, 512], F32, tag="ps512")
                    nc.tensor.matmul(y_ps[:, :], OT[:, :],
                                     wo[:, 512 * j:512 * (j + 1)],
                                     start=True, stop=True)
                    nc.vector.tensor_copy(y_attn[:, 512 * j:512 * (j + 1)], y_ps[:, :])
                all_reduce_add(y_attn, tag=f"a{bi}")

            def mlp_block(l):
                h = layer_norm(f"m{l}")
                hT = transpose_hT(h)

                a_ps = pps.tile([R, DFC], F32, tag="ps512")
                for kt_i in range(8):
                    w = wpool.tile([128, DFC], WDT, tag="w1")
                    nc.sync.dma_start(w[:, :], P[f'w1_{l}'][kt_i])
                    nc.tensor.matmul(a_ps[:, :],
                                     hT[:, 128 * kt_i:128 * (kt_i + 1)],
                                     w[:, :],
                                     start=(kt_i == 0), stop=(kt_i == 7))
                b1 = wpool.tile([R, DFC], F32, tag="b1")
                nc.sync.dma_start(b1[:, :], P[f'b1_{l}'][:, :])
                ab = qpool.tile([R, DFC], F32, tag="ab")
                nc.vector.tensor_add(ab[:, :], a_ps[:, :], b1[:, :])
                ag = qpool.tile([R, DFC], WDT, tag="ag")
                nc.scalar.activation(ag[:, :], ab[:, :],
                                     mybir.ActivationFunctionType.Gelu_apprx_tanh)
                aT = hpool.tile([128, DFC], WDT, tag="aT")
                for i in range(4):
                    transpose_128(ag[:, 128 * i:128 * (i + 1)],
                                  aT[:, 128 * i:128 * (i + 1)])

                y_ps = [pps.tile([R, 512], F32, tag="ps512", name=f"y2ps{j}")
                        for j in range(2)]
                for kt_i in range(4):
                    w = wpool.tile([128, D], WDT, tag="w2")
                    nc.sync.dma_start(w[:, :], P[f'w2_{l}'][kt_i])
                    for j in range(2):
                        nc.tensor.matmul(y_ps[j][:, :],
                                         aT[:, 128 * kt_i:128 * (kt_i + 1)],
                                         w[:, 512 * j:512 * (j + 1)],
                                         start=(kt_i == 0), stop=(kt_i == 3))
                b2 = wpool.tile([R, D], F32, tag="b2")
                nc.sync.dma_start(b2[:, :], P[f'b2_{l}'][:, :])
                y2 = qpool.tile([R, D], BF16, tag="y2")
                for j in range(2):
                    nc.vector.scalar_tensor_tensor(
                        y2[:, 512 * j:512 * (j + 1)], y_ps[j][:, :], 1.0,
                        b2[:, 512 * j:512 * (j + 1)],
                        op0=mybir.AluOpType.mult, op1=mybir.AluOpType.add)
                all_reduce_add(y2, tag=f"m{l}")

            for l in range(NL):
                attn_block(2 * l)
                attn_block(2 * l + 1)
                mlp_block(l)

            nc.sync.dma_start(out[:, :], x_sb[:, :])

    nc.compile()
    return nc


_cached_nc = None


def _get_nc():
    global _cached_nc
    if _cached_nc is None:
        _cached_nc = _build()
    return _cached_nc


def _run(inputs, trace=False):
    nc = _get_nc()
    in_maps = _prep_in_maps(inputs)
    res = run_bass_kernel_spmd(nc, in_maps, list(range(N_CORES)), trace=trace)
    y = res.results[0]['out'].reshape(B, L, D).astype(np.float32)
    return y, res


def kernel(**inputs):
    y, _ = _run(inputs, trace=False)
    return y


# revision 17
# speedup vs baseline: 1.3466x; 1.0047x over previous
"""PoET transformer-with-KV-prefix kernel for 8 Trainium2 NeuronCores.

Sharding: tensor-parallel over heads (2 heads/core) for attention and over
FFN columns (512/core) for the MLP.  Activations [B*L=128, D=1024] are
replicated; each block ends in an 8-core AllReduce (bf16) of the output
projection partial sums.  LayerNorm gains/biases are folded into the
following weight matrices host-side, so on-device LN is a pure normalize.

Attention scores are computed pre-transposed (K-tile stationary, q moving),
so exp() writes A^T directly and A@V needs no transposes.  The V tiles
carry a ones-column so the A@V accumulation also produces the softmax
denominator for free.
"""

import sys
import numpy as np

for _p in ("/opt/trn_rl_repo", "/root/.axon_site/_ro/trn_rl_repo"):
    if _p not in sys.path:
        sys.path.insert(0, _p)

import ml_dtypes
import concourse.bass as bass
import concourse.bacc as bacc
import concourse.mybir as mybir
from concourse.tile import TileContext
from concourse.bass_utils import run_bass_kernel_spmd

# Problem dims (hardcoded per spec)
NL, B, L, D, H, Dh, S, DF = 2, 8, 16, 1024, 16, 64, 2048, 4096
ROPE_BASE = 10000.0
LN_EPS = 1e-5

N_CORES = 8
R = B * L            # 128 token rows
HPC = H // N_CORES   # 2 heads per core
FPC = HPC * Dh       # 128 features per core
DFC = DF // N_CORES  # 512 ffn cols per core
NT_PRE = S // 128    # 16 prefix t-tiles
NT = NT_PRE + 1      # 17 t-tiles including the new-token tile

F32 = mybir.dt.float32
F32R = mybir.dt.float32r
BF16 = mybir.dt.bfloat16
NPBF = ml_dtypes.bfloat16
RG = [list(range(N_CORES))]

WARMUP_CC = False    # tiny AllGather at t=0 to absorb collective setup/skew
W_BF16 = True        # bf16 weights + bf16 activation-stationary matmuls
WDT = BF16 if W_BF16 else F32R
NPW = NPBF if W_BF16 else np.float32


# ---------------------------------------------------------------------------
# Host-side input prep: fold LN into weights, transpose KV, slice per core.
# ---------------------------------------------------------------------------

def _prep_in_maps(inp):
    f = lambda k: np.asarray(inp[k], dtype=np.float32)
    x = f('x').reshape(R, D)

    # rope tables (token-major): row r -> position S + r % L
    pos = (S + np.arange(R) % L).astype(np.float32)
    inv = ROPE_BASE ** (-np.arange(Dh // 2, dtype=np.float32) / (Dh // 2))
    ang = pos[:, None] * inv[None, :]              # [128, 32]
    cos32, sin32 = np.cos(ang), np.sin(ang)
    blk_cos = np.concatenate([cos32, cos32], 1)    # [128, 64]
    blk_ssin = np.concatenate([-sin32, sin32], 1)  # [128, 64]
    cos2 = np.tile(blk_cos, (1, 4)).astype(np.float32)    # [128, 256] (q_h0,q_h1,k_h0,k_h1)
    ssin2 = np.tile(blk_ssin, (1, 4)).astype(np.float32)

    # block-diagonal own-batch mask for the new-token scores (symmetric)
    mask01 = np.kron(np.eye(B, dtype=np.float32),
                     np.ones((L, L), np.float32)).astype(NPBF)

    shared = {'x': x, 'cos2': cos2, 'ssin2': ssin2, 'mask01': mask01}

    attn_specs = [(0, 'self'), (0, 'cross'), (1, 'self'), (1, 'cross')]
    per_core = [dict(shared) for _ in range(N_CORES)]

    for bi, (l, kind) in enumerate(attn_specs):
        g = f('ln1_g' if kind == 'self' else 'ln2_g')[l]
        be = f('ln1_b' if kind == 'self' else 'ln2_b')[l]
        Wq, Wk, Wv, Wo = (f(f'{kind}_W{m}')[l] for m in 'qkvo')
        k_mem = f(f'{kind}_k_mem')[l]   # [S, H, Dh]
        v_mem = f(f'{kind}_v_mem')[l]
        Wq_e, Wk_e, Wv_e = g[:, None] * Wq, g[:, None] * Wk, g[:, None] * Wv
        bq, bk, bv = be @ Wq, be @ Wk, be @ Wv   # [D]
        for c in range(N_CORES):
            cs = slice(c * FPC, (c + 1) * FPC)
            wqkv = np.concatenate([Wq_e[:, cs], Wk_e[:, cs], Wv_e[:, cs]], 1)  # [1024, 384]
            bqkv = np.concatenate([bq[cs], bk[cs], bv[cs]])                    # [384]
            m = per_core[c]
            m[f'wqkv{bi}'] = np.ascontiguousarray(wqkv.reshape(8, 128, 3 * FPC)).astype(NPW)
            m[f'bqkv{bi}'] = np.ascontiguousarray(np.tile(bqkv[None, :], (128, 1)))
            m[f'wo{bi}'] = np.ascontiguousarray(Wo[cs, :]).astype(NPW)         # [128, 1024]
            # K^T per head, feature-major: [128 (2h x 64), S], bf16
            kt = k_mem[:, 2 * c:2 * c + 2, :].transpose(1, 2, 0).reshape(FPC, S)
            m[f'kt{bi}'] = np.ascontiguousarray(kt).astype(NPBF)
            # V token-major tiles with ones-columns: [128, 16*130], bf16
            # col layout per t-tile: [v_h0 (64) | 1 | v_h1 (64) | 1]
            v = v_mem[:, 2 * c:2 * c + 2, :].reshape(NT_PRE, 128, 2, Dh)
            va = np.ones((128, NT_PRE, 2, Dh + 1), np.float32)
            va[:, :, :, :Dh] = v.transpose(1, 0, 2, 3)
            m[f'v{bi}'] = np.ascontiguousarray(va.reshape(128, NT_PRE * 130)).astype(NPBF)

    for l in range(NL):
        g3, b3 = f('ln3_g')[l], f('ln3_b')[l]
        W1, b1, W2, b2 = f('W1')[l], f('b1')[l], f('W2')[l], f('b2')[l]
        W1_e = g3[:, None] * W1
        b1_e = b1 + b3 @ W1
        for c in range(N_CORES):
            cs = slice(c * DFC, (c + 1) * DFC)
            m = per_core[c]
            m[f'w1_{l}'] = np.ascontiguousarray(W1_e[:, cs].reshape(8, 128, DFC)).astype(NPW)
            m[f'b1_{l}'] = np.ascontiguousarray(np.tile(b1_e[None, cs], (128, 1)))
            m[f'w2_{l}'] = np.ascontiguousarray(W2[cs, :].reshape(4, 128, D)).astype(NPW)
            m[f'b2_{l}'] = np.ascontiguousarray(np.tile(b2[None, :] / N_CORES, (128, 1)))
    return per_core


# ---------------------------------------------------------------------------
# Device program (SPMD; identical on all cores, per-core data via in_maps)
# ---------------------------------------------------------------------------

def _build():
    from concourse import masks

    nc = bacc.Bacc("TRN2", target_bir_lowering=False, debug=False,
                   num_devices=N_CORES)
    P = {}
    P['x'] = nc.declare_dram_parameter('x', [R, D], F32, isOutput=False)
    P['cos2'] = nc.declare_dram_parameter('cos2', [R, 256], F32, isOutput=False)
    P['ssin2'] = nc.declare_dram_parameter('ssin2', [R, 256], F32, isOutput=False)
    P['mask01'] = nc.declare_dram_parameter('mask01', [R, R], BF16, isOutput=False)
    for bi in range(4):
        P[f'wqkv{bi}'] = nc.declare_dram_parameter(f'wqkv{bi}', [8, 128, 3 * FPC], WDT, isOutput=False)
        P[f'bqkv{bi}'] = nc.declare_dram_parameter(f'bqkv{bi}', [R, 3 * FPC], F32, isOutput=False)
        P[f'wo{bi}'] = nc.declare_dram_parameter(f'wo{bi}', [FPC, D], WDT, isOutput=False)
        P[f'kt{bi}'] = nc.declare_dram_parameter(f'kt{bi}', [FPC, S], BF16, isOutput=False)
        P[f'v{bi}'] = nc.declare_dram_parameter(f'v{bi}', [128, NT_PRE * 130], BF16, isOutput=False)
    for l in range(NL):
        P[f'w1_{l}'] = nc.declare_dram_parameter(f'w1_{l}', [8, 128, DFC], WDT, isOutput=False)
        P[f'b1_{l}'] = nc.declare_dram_parameter(f'b1_{l}', [R, DFC], F32, isOutput=False)
        P[f'w2_{l}'] = nc.declare_dram_parameter(f'w2_{l}', [4, 128, D], WDT, isOutput=False)
        P[f'b2_{l}'] = nc.declare_dram_parameter(f'b2_{l}', [R, D], F32, isOutput=False)
    out = nc.declare_dram_parameter('out', [R, D], F32, isOutput=True)

    with TileContext(nc) as tc:
        with (
            tc.tile_pool(name="cpool", bufs=1) as cpool,
            tc.tile_pool(name="hpool", bufs=2) as hpool,
            tc.tile_pool(name="qpool", bufs=2) as qpool,
            tc.tile_pool(name="apool", bufs=2) as apool,
            tc.tile_pool(name="kvpool", bufs=2) as kvpool,
            tc.tile_pool(name="wpool", bufs=3) as wpool,
            tc.tile_pool(name="spool", bufs=4) as spool,
            tc.tile_pool(name="ppt", bufs=2, space="PSUM") as ppt,
            tc.tile_pool(name="pps", bufs=4, space="PSUM") as pps,
            tc.tile_pool(name="ppa", bufs=2, space="PSUM") as ppa,
            tc.tile_pool(name="dpool", bufs=2, space="DRAM") as dpool,
        ):
            identb = cpool.tile([128, 128], BF16, tag="identb")
            masks.make_identity(nc, identb[:, :])
            x_sb = cpool.tile([R, D], F32, tag="x")
            nc.sync.dma_start(x_sb[:, :], P['x'][:, :])
            cos2 = cpool.tile([R, 256], F32, tag="cos2")
            nc.sync.dma_start(cos2[:, :], P['cos2'][:, :])
            ssin2 = cpool.tile([R, 256], F32, tag="ssin2")
            nc.sync.dma_start(ssin2[:, :], P['ssin2'][:, :])
            mask01 = cpool.tile([R, R], BF16, tag="mask01")
            nc.sync.dma_start(mask01[:, :], P['mask01'][:, :])
            eps_t = cpool.tile([128, 1], F32, tag="eps")
            nc.gpsimd.memset(eps_t[:, :], LN_EPS)

            if WARMUP_CC:
                wu_in = dpool.tile([2, 16], F32, tag="wu_in")
                wu_out = dpool.tile([16, 16], F32, tag="wu_out")
                nc.gpsimd.dma_start(wu_in[:], P['x'][0:2, 0:16])
                nc.gpsimd.collective_compute(
                    "AllGather", mybir.AluOpType.bypass, replica_groups=RG,
                    ins=[wu_in.opt()], outs=[wu_out.opt()])

            def layer_norm(tag):
                """x_sb -> hb [R, D] bf16 normalized (gains folded away)."""
                sums = spool.tile([R, 8], F32, tag="lnsums")
                nc.vector.tensor_reduce(sums[:, 0:1], x_sb[:, :],
                                        axis=mybir.AxisListType.X, op=mybir.AluOpType.add)
                sq = hpool.tile([R, D], F32, tag="sq")
                nc.scalar.activation(sq[:, :], x_sb[:, :],
                                     mybir.ActivationFunctionType.Square,
                                     accum_out=sums[:, 1:2])             # sum(x^2)
                nc.vector.tensor_scalar(sums[:, 2:3], sums[:, 0:1], 1.0 / D,
                                        None, op0=mybir.AluOpType.mult)  # mean
                nc.vector.tensor_tensor(sums[:, 3:4], sums[:, 2:3], sums[:, 2:3],
                                        op=mybir.AluOpType.mult)         # mean^2
                nc.vector.tensor_scalar(sums[:, 4:5], sums[:, 3:4], -1.0,
                                        eps_t[:, 0:1], op0=mybir.AluOpType.mult,
                                        op1=mybir.AluOpType.add)         # eps - mean^2
                nc.scalar.activation(sums[:, 5:6], sums[:, 1:2],
                                     mybir.ActivationFunctionType.Sqrt,
                                     scale=1.0 / D, bias=sums[:, 4:5])   # sqrt(var+eps)
                nc.vector.reciprocal(sums[:, 6:7], sums[:, 5:6])         # rstd
                h = hpool.tile([R, D], WDT, tag="h")
                nc.vector.tensor_scalar(h[:, :], x_sb[:, :], sums[:, 2:3], sums[:, 6:7],
                                        op0=mybir.AluOpType.subtract,
                                        op1=mybir.AluOpType.mult)
                return h

            def transpose_128(src_ap, dst_ap):
                """PE-transpose one bf16 [128, <=128] slice into SBUF dst."""
                np_, nf = src_ap.shape[0], src_ap.shape[1]
                p = ppt.tile([128, 128], BF16, tag="tpb")
                nc.tensor.transpose(p[:nf, :np_], src_ap, identb[:np_, :np_])
                nc.vector.tensor_copy(dst_ap, p[:nf, :np_])

            def transpose_hT(h, n=8):
                hT = hpool.tile([R, D], WDT, tag="hT")
                for i in range(n):
                    transpose_128(h[:, 128 * i:128 * (i + 1)],
                                  hT[:, 128 * i:128 * (i + 1)])
                return hT

            def all_reduce_add(y_sb, tag=""):
                """AllReduce (bf16) the [R, D] partial and add into x_sb."""
                cin = dpool.tile([R, D], BF16, tag="cc_in")
                cout = dpool.tile([R, D], BF16, tag="cc_out")
                nc.gpsimd.dma_start(cin[:, :], y_sb[:, :])
                nc.gpsimd.collective_compute(
                    "AllReduce", mybir.AluOpType.add, replica_groups=RG,
                    ins=[cin.opt()], outs=[cout.opt()])
                y = hpool.tile([R, D], BF16, tag="yred")
                nc.sync.dma_start(y[:, :], cout[:, :])
                nc.vector.tensor_add(x_sb[:, :], x_sb[:, :], y[:, :])

            def attn_block(bi):
                # prefix K^T and V (prefetchable, no deps)
                kt_sb = kvpool.tile([FPC, S], BF16, tag="kt")
                nc.sync.dma_start(kt_sb[:, :], P[f'kt{bi}'][:, :])
                v_sb = kvpool.tile([128, NT_PRE * 130], BF16, tag="v")
                nc.sync.dma_start(v_sb[:, :], P[f'v{bi}'][:, :])

                h = layer_norm(f"a{bi}")
                hT = transpose_hT(h)

                # qkv = h @ Wqkv_c + bqkv   [R, 384] (token-major)
                qkv_ps = pps.tile([R, 3 * FPC], F32, tag="ps512")
                for kt_i in range(8):
                    w = wpool.tile([128, 3 * FPC], WDT, tag="wqkv")
                    nc.sync.dma_start(w[:, :], P[f'wqkv{bi}'][kt_i])
                    nc.tensor.matmul(qkv_ps[:, :],
                                     hT[:, 128 * kt_i:128 * (kt_i + 1)],
                                     w[:, :],
                                     start=(kt_i == 0), stop=(kt_i == 7))
                bq = wpool.tile([R, 3 * FPC], F32, tag="bqkv")
                nc.sync.dma_start(bq[:, :], P[f'bqkv{bi}'][:, :])
                qkv = qpool.tile([R, 3 * FPC], F32, tag="qkv")
                nc.vector.tensor_add(qkv[:, :], qkv_ps[:, :], bq[:, :])

                # rope on q|k region [R, 256] -> bf16
                tmp = qpool.tile([R, 256], F32, tag="ropetmp")
                for blk in range(4):
                    a0, a1, a2 = 64 * blk, 64 * blk + 32, 64 * blk + 64
                    nc.vector.tensor_mul(tmp[:, a0:a1], qkv[:, a1:a2], ssin2[:, a0:a1])
                    nc.vector.tensor_mul(tmp[:, a1:a2], qkv[:, a0:a1], ssin2[:, a1:a2])
                qk_c = qpool.tile([R, 256], F32, tag="qkc")
                nc.vector.tensor_mul(qk_c[:, :], qkv[:, 0:256], cos2[:, :])
                qk_r = qpool.tile([R, 256], BF16, tag="qkr")
                nc.vector.tensor_add(qk_r[:, :], qk_c[:, :], tmp[:, :])

                qT = qpool.tile([FPC, R], BF16, tag="qT")
                transpose_128(qk_r[:, 0:128], qT[:, :])
                kTn = qpool.tile([FPC, R], BF16, tag="kTn")
                transpose_128(qk_r[:, 128:256], kTn[:, :])

                # new-token V with ones columns, bf16: [v_h0|1|v_h1|1]
                vn = qpool.tile([128, 130], BF16, tag="vn")
                nc.vector.tensor_copy(vn[:, 0:Dh], qkv[:, 256:256 + Dh])
                nc.gpsimd.memset(vn[:, Dh:Dh + 1], 1.0)
                nc.vector.tensor_copy(vn[:, Dh + 1:2 * Dh + 1], qkv[:, 256 + Dh:256 + 2 * Dh])
                nc.gpsimd.memset(vn[:, 2 * Dh + 1:2 * Dh + 2], 1.0)

                O = qpool.tile([R, FPC], BF16, tag="O")
                inv_sqrt_d = 1.0 / np.sqrt(Dh)
                # Both heads interleaved: scores pre-transposed (K-tile
                # stationary, q moving) so exp writes A^T directly; A@V
                # accumulates with the ones-column giving the softmax sum.
                hslices = [slice(Dh * hh, Dh * (hh + 1)) for hh in range(HPC)]
                ATs = [apool.tile([128, NT * 128], BF16, tag="AT", name=f"AT{bi}_{hh}")
                       for hh in range(HPC)]
                avs = [ppa.tile([R, Dh + 1], F32, tag="av", name=f"av{bi}_{hh}")
                       for hh in range(HPC)]
                # Software pipeline (depth 2): scores(j) run while exp(j-1)
                # finishes, A@V(j-1) follows -- PE never stalls on ACT.
                def emit_scores(j):
                    sps = []
                    for hh in range(HPC):
                        s_ps = pps.tile([R, 512], F32, tag="ps512",
                                        name=f"sps{hh}")
                        for tt in range(4):
                            t = 4 * j + tt
                            nc.tensor.matmul(s_ps[:, 128 * tt:128 * (tt + 1)],
                                             kt_sb[hslices[hh], 128 * t:128 * (t + 1)],
                                             qT[hslices[hh], :], start=True, stop=True)
                        sps.append(s_ps)
                    return sps

                def emit_exp(j, sps):
                    for hh in range(HPC):
                        nc.scalar.activation(ATs[hh][:, 512 * j:512 * (j + 1)],
                                             sps[hh][:, :],
                                             mybir.ActivationFunctionType.Exp,
                                             scale=inv_sqrt_d)

                def emit_av(j):
                    for hh in range(HPC):
                        for tt in range(4):
                            t = 4 * j + tt
                            nc.tensor.matmul(
                                avs[hh][:, :],
                                ATs[hh][:, 128 * t:128 * (t + 1)],
                                v_sb[:, 130 * t + 65 * hh: 130 * t + 65 * hh + 65],
                                start=(t == 0), stop=False)

                sps_q = {}
                for j in range(5):
                    if j < 4:
                        sps_q[j] = emit_scores(j)
                    if j >= 1:
                        emit_exp(j - 1, sps_q.pop(j - 1))
                        emit_av(j - 1)
                # new-token scores (transposed, [new_tok, row]), masked
                for hh in range(HPC):
                    sn_ps = ppt.tile([128, 128], F32, tag="tpb", name=f"snp{hh}")
                    nc.tensor.matmul(sn_ps[:, :], kTn[hslices[hh], :],
                                     qT[hslices[hh], :], start=True, stop=True)
                    en = qpool.tile([R, R], BF16, tag="expn", name=f"en{hh}")
                    nc.scalar.activation(en[:, :], sn_ps[:, :],
                                         mybir.ActivationFunctionType.Exp,
                                         scale=inv_sqrt_d)
                    nc.vector.tensor_mul(ATs[hh][:, S:S + R], en[:, :], mask01[:, :])
                    nc.tensor.matmul(avs[hh][:, :], ATs[hh][:, S:S + R],
                                     vn[:, 65 * hh: 65 * hh + 65],
                                     start=False, stop=True)
                sums = spool.tile([R, 2], F32, tag="smsums")
                for hh in range(HPC):
                    nc.vector.reciprocal(sums[:, hh:hh + 1], avs[hh][:, Dh:Dh + 1])
                    nc.vector.tensor_scalar(O[:, Dh * hh:Dh * (hh + 1)],
                                            avs[hh][:, 0:Dh],
                                            sums[:, hh:hh + 1], None,
                                            op0=mybir.AluOpType.mult)

                OT = qpool.tile([FPC, R], WDT, tag="OT")
                transpose_128(O[:, :], OT[:, :])
                wo = wpool.tile([FPC, D], WDT, tag="wo")
                nc.sync.dma_start(wo[:, :], P[f'wo{bi}'][:, :])
                y_attn = qpool.tile([R, D], BF16, tag="y2")
                for j in range(2):
                    y_ps = pps.tile([R, 512], F32, tag="ps512")
                    nc.tensor.matmul(y_ps[:, :], OT[:, :],
                                     wo[:, 512 * j:512 * (j + 1)],
                                     start=True, stop=True)
                    nc.vector.tensor_copy(y_attn[:, 512 * j:512 * (j + 1)], y_ps[:, :])
                all_reduce_add(y_attn, tag=f"a{bi}")

            def mlp_block(l):
                h = layer_norm(f"m{l}")
                hT = transpose_hT(h)

                a_ps = pps.tile([R, DFC], F32, tag="ps512")
                for kt_i in range(8):
                    w = wpool.tile([128, DFC], WDT, tag="w1")
                    nc.sync.dma_start(w[:, :], P[f'w1_{l}'][kt_i])
                    nc.tensor.matmul(a_ps[:, :],
                                     hT[:, 128 * kt_i:128 * (kt_i + 1)],
                                     w[:, :],
                                     start=(kt_i == 0), stop=(kt_i == 7))
                b1 = wpool.tile([R, DFC], F32, tag="b1")
                nc.sync.dma_start(b1[:, :], P[f'b1_{l}'][:, :])
                # bias+gelu+transpose+y2 pipelined per 128-col chunk of a
                ab = qpool.tile([R, DFC], F32, tag="ab")
                ag = qpool.tile([R, DFC], WDT, tag="ag")
                aT = hpool.tile([128, DFC], WDT, tag="aT")
                y_ps = [pps.tile([R, 512], F32, tag="ps512", name=f"y2ps{j}")
                        for j in range(2)]
                for i in range(4):
                    w2 = wpool.tile([128, D], WDT, tag="w2")
                    nc.sync.dma_start(w2[:, :], P[f'w2_{l}'][i])
                    cs = slice(128 * i, 128 * (i + 1))
                    nc.vector.tensor_add(ab[:, cs], a_ps[:, cs], b1[:, cs])
                    nc.scalar.activation(ag[:, cs], ab[:, cs],
                                         mybir.ActivationFunctionType.Gelu_apprx_tanh)
                    transpose_128(ag[:, cs], aT[:, cs])
                    for j in range(2):
                        nc.tensor.matmul(y_ps[j][:, :], aT[:, cs],
                                         w2[:, 512 * j:512 * (j + 1)],
                                         start=(i == 0), stop=(i == 3))
                b2 = wpool.tile([R, D], F32, tag="b2")
                nc.sync.dma_start(b2[:, :], P[f'b2_{l}'][:, :])
                y2 = qpool.tile([R, D], BF16, tag="y2")
                for j in range(2):
                    nc.vector.scalar_tensor_tensor(
                        y2[:, 512 * j:512 * (j + 1)], y_ps[j][:, :], 1.0,
                        b2[:, 512 * j:512 * (j + 1)],
                        op0=mybir.AluOpType.mult, op1=mybir.AluOpType.add)
                all_reduce_add(y2, tag=f"m{l}")

            for l in range(NL):
                attn_block(2 * l)
                attn_block(2 * l + 1)
                mlp_block(l)

            nc.sync.dma_start(out[:, :], x_sb[:, :])

    nc.compile()
    return nc


_cached_nc = None


def _get_nc():
    global _cached_nc
    if _cached_nc is None:
        _cached_nc = _build()
    return _cached_nc


def _run(inputs, trace=False):
    nc = _get_nc()
    in_maps = _prep_in_maps(inputs)
    res = run_bass_kernel_spmd(nc, in_maps, list(range(N_CORES)), trace=trace)
    y = res.results[0]['out'].reshape(B, L, D).astype(np.float32)
    return y, res


def kernel(**inputs):
    y, _ = _run(inputs, trace=False)
    return y


# revision 19
# speedup vs baseline: 1.5444x; 1.1469x over previous
"""PoET transformer-with-KV-prefix kernel for 8 Trainium2 NeuronCores.

Sharding: tensor-parallel over heads (2 heads/core) for attention and over
FFN columns (512/core) for the MLP.  Activations [B*L=128, D=1024] are
replicated; each block ends in an 8-core AllReduce (bf16) of the output
projection partial sums.  LayerNorm gains/biases are folded into the
following weight matrices host-side, so on-device LN is a pure normalize.

Attention scores are computed pre-transposed (K-tile stationary, q moving),
so exp() writes A^T directly and A@V needs no transposes.  The V tiles
carry a ones-column so the A@V accumulation also produces the softmax
denominator for free.
"""

import sys
import numpy as np

for _p in ("/opt/trn_rl_repo", "/root/.axon_site/_ro/trn_rl_repo"):
    if _p not in sys.path:
        sys.path.insert(0, _p)

import ml_dtypes
import concourse.bass as bass
import concourse.bacc as bacc
import concourse.mybir as mybir
from concourse.tile import TileContext
from concourse.bass_utils import run_bass_kernel_spmd

# Problem dims (hardcoded per spec)
NL, B, L, D, H, Dh, S, DF = 2, 8, 16, 1024, 16, 64, 2048, 4096
ROPE_BASE = 10000.0
LN_EPS = 1e-5

N_CORES = 8
R = B * L            # 128 token rows
HPC = H // N_CORES   # 2 heads per core
FPC = HPC * Dh       # 128 features per core
DFC = DF // N_CORES  # 512 ffn cols per core
NT_PRE = S // 128    # 16 prefix t-tiles
NT = NT_PRE + 1      # 17 t-tiles including the new-token tile

F32 = mybir.dt.float32
F32R = mybir.dt.float32r
BF16 = mybir.dt.bfloat16
NPBF = ml_dtypes.bfloat16
RG = [list(range(N_CORES))]

WARMUP_CC = False    # tiny AllGather at t=0 to absorb collective setup/skew
W_BF16 = True        # bf16 weights + bf16 activation-stationary matmuls
WDT = BF16 if W_BF16 else F32R
NPW = NPBF if W_BF16 else np.float32


# ---------------------------------------------------------------------------
# Host-side input prep: fold LN into weights, transpose KV, slice per core.
# ---------------------------------------------------------------------------

def _prep_in_maps(inp):
    f = lambda k: np.asarray(inp[k], dtype=np.float32)
    x = f('x').reshape(R, D)

    # rope tables (token-major): row r -> position S + r % L
    pos = (S + np.arange(R) % L).astype(np.float32)
    inv = ROPE_BASE ** (-np.arange(Dh // 2, dtype=np.float32) / (Dh // 2))
    ang = pos[:, None] * inv[None, :]              # [128, 32]
    cos32, sin32 = np.cos(ang), np.sin(ang)
    blk_cos = np.concatenate([cos32, cos32], 1)    # [128, 64]
    blk_ssin = np.concatenate([-sin32, sin32], 1)  # [128, 64]
    cos2 = np.tile(blk_cos, (1, 4)).astype(np.float32)    # [128, 256] (q_h0,q_h1,k_h0,k_h1)
    ssin2 = np.tile(blk_ssin, (1, 4)).astype(np.float32)

    # block-diagonal own-batch mask for the new-token scores (symmetric)
    mask01 = np.kron(np.eye(B, dtype=np.float32),
                     np.ones((L, L), np.float32)).astype(NPBF)

    shared = {'x': x, 'cos2': cos2, 'ssin2': ssin2, 'mask01': mask01}

    attn_specs = [(0, 'self'), (0, 'cross'), (1, 'self'), (1, 'cross')]
    per_core = [dict(shared) for _ in range(N_CORES)]

    for bi, (l, kind) in enumerate(attn_specs):
        g = f('ln1_g' if kind == 'self' else 'ln2_g')[l]
        be = f('ln1_b' if kind == 'self' else 'ln2_b')[l]
        Wq, Wk, Wv, Wo = (f(f'{kind}_W{m}')[l] for m in 'qkvo')
        k_mem = f(f'{kind}_k_mem')[l]   # [S, H, Dh]
        v_mem = f(f'{kind}_v_mem')[l]
        Wq_e, Wk_e, Wv_e = g[:, None] * Wq, g[:, None] * Wk, g[:, None] * Wv
        bq, bk, bv = be @ Wq, be @ Wk, be @ Wv   # [D]
        for c in range(N_CORES):
            cs = slice(c * FPC, (c + 1) * FPC)
            wqkv = np.concatenate([Wq_e[:, cs], Wk_e[:, cs], Wv_e[:, cs]], 1)  # [1024, 384]
            bqkv = np.concatenate([bq[cs], bk[cs], bv[cs]])                    # [384]
            m = per_core[c]
            m[f'wqkv{bi}'] = np.ascontiguousarray(wqkv.reshape(8, 128, 3 * FPC)).astype(NPW)
            m[f'bqkv{bi}'] = np.ascontiguousarray(np.tile(bqkv[None, :], (128, 1)))
            m[f'wo{bi}'] = np.ascontiguousarray(Wo[cs, :]).astype(NPW)         # [128, 1024]
            # K^T per head, feature-major: [128 (2h x 64), S], bf16
            kt = k_mem[:, 2 * c:2 * c + 2, :].transpose(1, 2, 0).reshape(FPC, S)
            m[f'kt{bi}'] = np.ascontiguousarray(kt).astype(NPBF)
            # V token-major tiles with ones-columns: [128, 16*130], bf16
            # col layout per t-tile: [v_h0 (64) | 1 | v_h1 (64) | 1]
            v = v_mem[:, 2 * c:2 * c + 2, :].reshape(NT_PRE, 128, 2, Dh)
            va = np.ones((128, NT_PRE, 2, Dh + 1), np.float32)
            va[:, :, :, :Dh] = v.transpose(1, 0, 2, 3)
            m[f'v{bi}'] = np.ascontiguousarray(va.reshape(128, NT_PRE * 130)).astype(NPBF)

    for l in range(NL):
        g3, b3 = f('ln3_g')[l], f('ln3_b')[l]
        W1, b1, W2, b2 = f('W1')[l], f('b1')[l], f('W2')[l], f('b2')[l]
        W1_e = g3[:, None] * W1
        b1_e = b1 + b3 @ W1
        for c in range(N_CORES):
            cs = slice(c * DFC, (c + 1) * DFC)
            m = per_core[c]
            m[f'w1_{l}'] = np.ascontiguousarray(W1_e[:, cs].reshape(8, 128, DFC)).astype(NPW)
            m[f'b1_{l}'] = np.ascontiguousarray(np.tile(b1_e[None, cs], (128, 1)))
            m[f'w2_{l}'] = np.ascontiguousarray(W2[cs, :].reshape(4, 128, D)).astype(NPW)
            m[f'b2_{l}'] = np.ascontiguousarray(np.tile(b2[None, :] / N_CORES, (128, 1)))
    return per_core


# ---------------------------------------------------------------------------
# Device program (SPMD; identical on all cores, per-core data via in_maps)
# ---------------------------------------------------------------------------

def _build():
    from concourse import masks

    nc = bacc.Bacc("TRN2", target_bir_lowering=False, debug=False,
                   num_devices=N_CORES)
    P = {}
    P['x'] = nc.declare_dram_parameter('x', [R, D], F32, isOutput=False)
    P['cos2'] = nc.declare_dram_parameter('cos2', [R, 256], F32, isOutput=False)
    P['ssin2'] = nc.declare_dram_parameter('ssin2', [R, 256], F32, isOutput=False)
    P['mask01'] = nc.declare_dram_parameter('mask01', [R, R], BF16, isOutput=False)
    for bi in range(4):
        P[f'wqkv{bi}'] = nc.declare_dram_parameter(f'wqkv{bi}', [8, 128, 3 * FPC], WDT, isOutput=False)
        P[f'bqkv{bi}'] = nc.declare_dram_parameter(f'bqkv{bi}', [R, 3 * FPC], F32, isOutput=False)
        P[f'wo{bi}'] = nc.declare_dram_parameter(f'wo{bi}', [FPC, D], WDT, isOutput=False)
        P[f'kt{bi}'] = nc.declare_dram_parameter(f'kt{bi}', [FPC, S], BF16, isOutput=False)
        P[f'v{bi}'] = nc.declare_dram_parameter(f'v{bi}', [128, NT_PRE * 130], BF16, isOutput=False)
    for l in range(NL):
        P[f'w1_{l}'] = nc.declare_dram_parameter(f'w1_{l}', [8, 128, DFC], WDT, isOutput=False)
        P[f'b1_{l}'] = nc.declare_dram_parameter(f'b1_{l}', [R, DFC], F32, isOutput=False)
        P[f'w2_{l}'] = nc.declare_dram_parameter(f'w2_{l}', [4, 128, D], WDT, isOutput=False)
        P[f'b2_{l}'] = nc.declare_dram_parameter(f'b2_{l}', [R, D], F32, isOutput=False)
    out = nc.declare_dram_parameter('out', [R, D], F32, isOutput=True)

    with TileContext(nc) as tc:
        with (
            tc.tile_pool(name="cpool", bufs=1) as cpool,
            tc.tile_pool(name="hpool", bufs=2) as hpool,
            tc.tile_pool(name="qpool", bufs=2) as qpool,
            tc.tile_pool(name="apool", bufs=2) as apool,
            tc.tile_pool(name="kvpool", bufs=2) as kvpool,
            tc.tile_pool(name="wpool", bufs=8) as wpool,
            tc.tile_pool(name="wpool4", bufs=4) as wpool4,
            tc.tile_pool(name="wpool2", bufs=2) as wpool2,
            tc.tile_pool(name="spool", bufs=4) as spool,
            tc.tile_pool(name="ppt", bufs=2, space="PSUM") as ppt,
            tc.tile_pool(name="pps", bufs=4, space="PSUM") as pps,
            tc.tile_pool(name="ppa", bufs=2, space="PSUM") as ppa,
            tc.tile_pool(name="dpool", bufs=2, space="DRAM") as dpool,
        ):
            identb = cpool.tile([128, 128], BF16, tag="identb")
            masks.make_identity(nc, identb[:, :])
            x_sb = cpool.tile([R, D], F32, tag="x")
            nc.sync.dma_start(x_sb[:, :], P['x'][:, :])
            cos2 = cpool.tile([R, 256], F32, tag="cos2")
            nc.sync.dma_start(cos2[:, :], P['cos2'][:, :])
            ssin2 = cpool.tile([R, 256], F32, tag="ssin2")
            nc.sync.dma_start(ssin2[:, :], P['ssin2'][:, :])
            mask01 = cpool.tile([R, R], BF16, tag="mask01")
            nc.sync.dma_start(mask01[:, :], P['mask01'][:, :])
            eps_t = cpool.tile([128, 1], F32, tag="eps")
            nc.gpsimd.memset(eps_t[:, :], LN_EPS)

            if WARMUP_CC:
                wu_in = dpool.tile([2, 16], F32, tag="wu_in")
                wu_out = dpool.tile([16, 16], F32, tag="wu_out")
                nc.gpsimd.dma_start(wu_in[:], P['x'][0:2, 0:16])
                nc.gpsimd.collective_compute(
                    "AllGather", mybir.AluOpType.bypass, replica_groups=RG,
                    ins=[wu_in.opt()], outs=[wu_out.opt()])

            def layer_norm(tag):
                """x_sb -> h [R, D] normalized (gains folded away)."""
                stats = spool.tile([R, 12], F32, tag="lnstats")
                aggr = spool.tile([R, 4], F32, tag="lnaggr")
                for g in range(2):
                    nc.vector.bn_stats(stats[:, 6 * g:6 * (g + 1)],
                                       x_sb[:, 512 * g:512 * (g + 1)])
                nc.vector.bn_aggr(aggr[:, 0:2], stats[:, :])
                nc.scalar.activation(aggr[:, 2:3], aggr[:, 1:2],
                                     mybir.ActivationFunctionType.Sqrt,
                                     bias=eps_t[:, 0:1])                 # sqrt(var+eps)
                nc.vector.reciprocal(aggr[:, 3:4], aggr[:, 2:3])         # rstd
                h = hpool.tile([R, D], WDT, tag="h")
                nc.vector.tensor_scalar(h[:, :], x_sb[:, :], aggr[:, 0:1], aggr[:, 3:4],
                                        op0=mybir.AluOpType.subtract,
                                        op1=mybir.AluOpType.mult)
                return h

            def transpose_128(src_ap, dst_ap):
                """PE-transpose one bf16 [128, <=128] slice into SBUF dst."""
                np_, nf = src_ap.shape[0], src_ap.shape[1]
                p = ppt.tile([128, 128], BF16, tag="tpb")
                nc.tensor.transpose(p[:nf, :np_], src_ap, identb[:np_, :np_])
                nc.vector.tensor_copy(dst_ap, p[:nf, :np_])

            def transpose_hT(h, n=8):
                hT = hpool.tile([R, D], WDT, tag="hT")
                for i in range(n):
                    transpose_128(h[:, 128 * i:128 * (i + 1)],
                                  hT[:, 128 * i:128 * (i + 1)])
                return hT

            def all_reduce_add(y_sb, tag="", chunks=1):
                """AllReduce (bf16) the [R, D] partial and add into x_sb."""
                cin = dpool.tile([R, D], BF16, tag="cc_in")
                cout = dpool.tile([R, D], BF16, tag="cc_out")
                for ch in range(chunks):
                    w = D // chunks
                    nc.gpsimd.dma_start(cin[:, ch * w:(ch + 1) * w],
                                        y_sb[:, ch * w:(ch + 1) * w])
                nc.gpsimd.collective_compute(
                    "AllReduce", mybir.AluOpType.add, replica_groups=RG,
                    ins=[cin.opt()], outs=[cout.opt()])
                y = hpool.tile([R, D], BF16, tag="yred")
                nc.sync.dma_start(y[:, :], cout[:, :])
                nc.vector.tensor_add(x_sb[:, :], x_sb[:, :], y[:, :])

            def attn_block(bi):
                # prefix K^T and V (prefetchable, no deps)
                kt_sb = kvpool.tile([FPC, S], BF16, tag="kt")
                nc.sync.dma_start(kt_sb[:, :], P[f'kt{bi}'][:, :])
                v_sb = kvpool.tile([128, NT_PRE * 130], BF16, tag="v")
                nc.sync.dma_start(v_sb[:, :], P[f'v{bi}'][:, :])

                h = layer_norm(f"a{bi}")
                hT = transpose_hT(h)

                # qkv = h @ Wqkv_c + bqkv   [R, 384] (token-major)
                qkv_ps = pps.tile([R, 3 * FPC], F32, tag="ps512")
                for kt_i in range(8):
                    w = wpool.tile([128, 3 * FPC], WDT, tag="wqkv")
                    nc.sync.dma_start(w[:, :], P[f'wqkv{bi}'][kt_i])
                    nc.tensor.matmul(qkv_ps[:, :],
                                     hT[:, 128 * kt_i:128 * (kt_i + 1)],
                                     w[:, :],
                                     start=(kt_i == 0), stop=(kt_i == 7))
                bq = wpool2.tile([R, 3 * FPC], F32, tag="bqkv")
                nc.sync.dma_start(bq[:, :], P[f'bqkv{bi}'][:, :])
                qkv = qpool.tile([R, 3 * FPC], F32, tag="qkv")
                nc.vector.tensor_add(qkv[:, :], qkv_ps[:, :], bq[:, :])

                # rope on q|k region [R, 256] -> bf16, q half first so its
                # transpose (and the score matmuls) can start early
                tmp = qpool.tile([R, 256], F32, tag="ropetmp")
                qk_c = qpool.tile([R, 256], F32, tag="qkc")
                qk_r = qpool.tile([R, 256], BF16, tag="qkr")
                qT = qpool.tile([FPC, R], BF16, tag="qT")
                kTn = qpool.tile([FPC, R], BF16, tag="kTn")
                for half, dst in ((0, qT), (1, kTn)):
                    o = 128 * half
                    for blk in range(2):
                        a0, a1, a2 = o + 64 * blk, o + 64 * blk + 32, o + 64 * blk + 64
                        nc.vector.tensor_mul(tmp[:, a0:a1], qkv[:, a1:a2], ssin2[:, a0:a1])
                        nc.vector.tensor_mul(tmp[:, a1:a2], qkv[:, a0:a1], ssin2[:, a1:a2])
                    nc.vector.tensor_mul(qk_c[:, o:o + 128], qkv[:, o:o + 128],
                                         cos2[:, o:o + 128])
                    nc.vector.tensor_add(qk_r[:, o:o + 128], qk_c[:, o:o + 128],
                                         tmp[:, o:o + 128])
                    transpose_128(qk_r[:, o:o + 128], dst[:, :])

                # new-token V with ones columns, bf16: [v_h0|1|v_h1|1]
                vn = qpool.tile([128, 130], BF16, tag="vn")
                nc.vector.tensor_copy(vn[:, 0:Dh], qkv[:, 256:256 + Dh])
                nc.gpsimd.memset(vn[:, Dh:Dh + 1], 1.0)
                nc.vector.tensor_copy(vn[:, Dh + 1:2 * Dh + 1], qkv[:, 256 + Dh:256 + 2 * Dh])
                nc.gpsimd.memset(vn[:, 2 * Dh + 1:2 * Dh + 2], 1.0)

                O = qpool.tile([R, FPC], BF16, tag="O")
                inv_sqrt_d = 1.0 / np.sqrt(Dh)
                # Both heads interleaved: scores pre-transposed (K-tile
                # stationary, q moving) so exp writes A^T directly; A@V
                # accumulates with the ones-column giving the softmax sum.
                hslices = [slice(Dh * hh, Dh * (hh + 1)) for hh in range(HPC)]
                ATs = [apool.tile([128, NT * 128], BF16, tag="AT", name=f"AT{bi}_{hh}")
                       for hh in range(HPC)]
                avs = [ppa.tile([R, Dh + 1], F32, tag="av", name=f"av{bi}_{hh}")
                       for hh in range(HPC)]
                # Software pipeline (depth 2): scores(j) run while exp(j-1)
                # finishes, A@V(j-1) follows -- PE never stalls on ACT.
                def emit_scores(j):
                    sps = []
                    for hh in range(HPC):
                        s_ps = pps.tile([R, 512], F32, tag="ps512",
                                        name=f"sps{hh}")
                        for tt in range(4):
                            t = 4 * j + tt
                            nc.tensor.matmul(s_ps[:, 128 * tt:128 * (tt + 1)],
                                             kt_sb[hslices[hh], 128 * t:128 * (t + 1)],
                                             qT[hslices[hh], :], start=True, stop=True)
                        sps.append(s_ps)
                    return sps

                def emit_exp(j, sps):
                    for hh in range(HPC):
                        nc.scalar.activation(ATs[hh][:, 512 * j:512 * (j + 1)],
                                             sps[hh][:, :],
                                             mybir.ActivationFunctionType.Exp,
                                             scale=inv_sqrt_d)

                def emit_av(j):
                    for hh in range(HPC):
                        for tt in range(4):
                            t = 4 * j + tt
                            nc.tensor.matmul(
                                avs[hh][:, :],
                                ATs[hh][:, 128 * t:128 * (t + 1)],
                                v_sb[:, 130 * t + 65 * hh: 130 * t + 65 * hh + 65],
                                start=(t == 0), stop=False)

                sps_q = {}
                for j in range(5):
                    if j < 4:
                        sps_q[j] = emit_scores(j)
                    if j >= 1:
                        emit_exp(j - 1, sps_q.pop(j - 1))
                        emit_av(j - 1)
                # new-token scores (transposed, [new_tok, row]), masked
                for hh in range(HPC):
                    sn_ps = ppt.tile([128, 128], F32, tag="tpb", name=f"snp{hh}")
                    nc.tensor.matmul(sn_ps[:, :], kTn[hslices[hh], :],
                                     qT[hslices[hh], :], start=True, stop=True)
                    en = qpool.tile([R, R], BF16, tag="expn", name=f"en{hh}")
                    nc.scalar.activation(en[:, :], sn_ps[:, :],
                                         mybir.ActivationFunctionType.Exp,
                                         scale=inv_sqrt_d)
                    nc.vector.tensor_mul(ATs[hh][:, S:S + R], en[:, :], mask01[:, :])
                    nc.tensor.matmul(avs[hh][:, :], ATs[hh][:, S:S + R],
                                     vn[:, 65 * hh: 65 * hh + 65],
                                     start=False, stop=True)
                sums = spool.tile([R, 2], F32, tag="smsums")
                for hh in range(HPC):
                    nc.vector.reciprocal(sums[:, hh:hh + 1], avs[hh][:, Dh:Dh + 1])
                    nc.vector.tensor_scalar(O[:, Dh * hh:Dh * (hh + 1)],
                                            avs[hh][:, 0:Dh],
                                            sums[:, hh:hh + 1], None,
                                            op0=mybir.AluOpType.mult)

                OT = qpool.tile([FPC, R], WDT, tag="OT")
                transpose_128(O[:, :], OT[:, :])
                wo = wpool2.tile([FPC, D], WDT, tag="wo")
                nc.sync.dma_start(wo[:, :], P[f'wo{bi}'][:, :])
                y_attn = qpool.tile([R, D], BF16, tag="y2")
                for j in range(2):
                    y_ps = pps.tile([R, 512], F32, tag="ps512")
                    nc.tensor.matmul(y_ps[:, :], OT[:, :],
                                     wo[:, 512 * j:512 * (j + 1)],
                                     start=True, stop=True)
                    nc.vector.tensor_copy(y_attn[:, 512 * j:512 * (j + 1)], y_ps[:, :])
                all_reduce_add(y_attn, tag=f"a{bi}", chunks=2)

            def mlp_block(l):
                h = layer_norm(f"m{l}")
                hT = transpose_hT(h)

                a_ps = pps.tile([R, DFC], F32, tag="ps512")
                for kt_i in range(8):
                    w = wpool.tile([128, DFC], WDT, tag="w1")
                    nc.sync.dma_start(w[:, :], P[f'w1_{l}'][kt_i])
                    nc.tensor.matmul(a_ps[:, :],
                                     hT[:, 128 * kt_i:128 * (kt_i + 1)],
                                     w[:, :],
                                     start=(kt_i == 0), stop=(kt_i == 7))
                b1 = wpool2.tile([R, DFC], F32, tag="b1")
                nc.sync.dma_start(b1[:, :], P[f'b1_{l}'][:, :])
                # bias+gelu+transpose+y2 pipelined per 128-col chunk of a
                ab = qpool.tile([R, DFC], F32, tag="ab")
                ag = qpool.tile([R, DFC], WDT, tag="ag")
                aT = hpool.tile([128, DFC], WDT, tag="aT")
                y_ps = [pps.tile([R, 512], F32, tag="ps512", name=f"y2ps{j}")
                        for j in range(2)]
                for i in range(4):
                    w2 = wpool4.tile([128, D], WDT, tag="w2")
                    nc.sync.dma_start(w2[:, :], P[f'w2_{l}'][i])
                    cs = slice(128 * i, 128 * (i + 1))
                    nc.vector.tensor_add(ab[:, cs], a_ps[:, cs], b1[:, cs])
                    nc.scalar.activation(ag[:, cs], ab[:, cs],
                                         mybir.ActivationFunctionType.Gelu_apprx_tanh)
                    transpose_128(ag[:, cs], aT[:, cs])
                    for j in range(2):
                        nc.tensor.matmul(y_ps[j][:, :], aT[:, cs],
                                         w2[:, 512 * j:512 * (j + 1)],
                                         start=(i == 0), stop=(i == 3))
                b2 = wpool2.tile([R, D], F32, tag="b2")
                nc.sync.dma_start(b2[:, :], P[f'b2_{l}'][:, :])
                y2 = qpool.tile([R, D], BF16, tag="y2")
                for j in range(2):
                    nc.vector.scalar_tensor_tensor(
                        y2[:, 512 * j:512 * (j + 1)], y_ps[j][:, :], 1.0,
                        b2[:, 512 * j:512 * (j + 1)],
                        op0=mybir.AluOpType.mult, op1=mybir.AluOpType.add)
                all_reduce_add(y2, tag=f"m{l}")

            for l in range(NL):
                attn_block(2 * l)
                attn_block(2 * l + 1)
                mlp_block(l)

            nc.sync.dma_start(out[:, :], x_sb[:, :])

    nc.compile()
    return nc


_cached_nc = None


def _get_nc():
    global _cached_nc
    if _cached_nc is None:
        _cached_nc = _build()
    return _cached_nc


def _run(inputs, trace=False):
    nc = _get_nc()
    in_maps = _prep_in_maps(inputs)
    res = run_bass_kernel_spmd(nc, in_maps, list(range(N_CORES)), trace=trace)
    y = res.results[0]['out'].reshape(B, L, D).astype(np.float32)
    return y, res


def kernel(**inputs):
    y, _ = _run(inputs, trace=False)
    return y
